# revision 1
# baseline (speedup 1.0000x reference)
"""Trainium2 Bass kernel for nn_DecoderBlock (B=8, N=1024, D=512, H=8, DH=64, DE=2048).

Strategy: 8-way data parallel over batch B — each NeuronCore computes the full
decoder block for one batch element; no collectives.

Algebraic refactors (exact in real arithmetic):
  1. Softmax-free attention is linear, so
         out @ W_merge = sum_h q_h @ (k_h^T @ h) @ (W_v_h @ W_merge_h)
     With M_h := W_v_h @ W_merge_h precomputed on host, the V projection and
     the merge matmul collapse into two small per-head matmuls through the
     64-dim head bottleneck. Assumes the v-slice of b_qkv is zero (true for
     this problem's setup_inputs).
  2. LN2's mean-centering is folded into W_ff1: subtracting the per-row
     column-mean from W_ff1 (and the mean from its bias) makes the ff1 matmul
     emit pre-centered activations, so LN2 only needs a sum-of-squares.

All matmuls run in float32r (full PE rate at N>=256, ~1e-4 rel accuracy);
f32 DRAM weights are DMA'd through bitcast-f32r views (no cast, HWDGE ok).

Low-priority "filler" matmul chains are emitted at the end of each phase so
the Tile scheduler drops them into PE idle slots — keeping the PE_HAM
activity monitor at full clock through DVE/ACT-bound stretches.

Device dataflow per core (seq-major = [seq on 128 partitions, feat], T = feat-major):
  x --LN0,+pos,swish--> h --PE-T--> hT
  qT = Wq^T hT (feat-major; score scale folded into Wq);  k = h Wk (seq-major)
  sT = h^T k   (all heads' s_h^T side by side);  sW_h = s_h M_h (64x512)
  x1 = x + sum_pairs qT_pair^T [sW_2j; sW_2j+1] + b_merge
  g1 = (x1-mu)*rstd --PE-T--> g1T          (LN1 gain/bias folded into W_ff1)
  per seq-half (pipelined):
    fTc = Wff1c^T g1T   (pre-centered);  var = mean(fTc^2) via ones-matmul
    f2T = silu(g2*fTc*rstd + b2);  y = x1 + f2T^T Wff2 + b_ff2
"""

import numpy as np

_B, _N, _D = 8, 1024, 512
_H, _DH, _DE = 8, 64, 2048
_EPS = 1e-5
_P = 128
_NT = _N // _P      # 8 seq chunks
_KD = _D // _P      # 4 d chunks
_KE = _DE // _P     # 16 d_expand chunks
_NCORES = 8


def _patch_tile_drain():
    """Walrus in this container caps sync-waits per TPB_CTRL instruction; the
    stock TileContext exit drain attaches one wait per live proc. Split the
    excess onto single-wait SP nops emitted before the semaphore reset."""
    import bass_rust
    import concourse.tile as tile

    if getattr(tile.TileContext, "_drain_patched", False):
        return

    def _drain_and_barrier(self, tick_clock, wait_clock):
        nc = self.nc
        drain_inst = nc.sync.drain()
        wait_clock.add_sem_waits(
            drain_inst.ins, tile.ScopedClock({None: tick_clock.global_clock})
        )
        si = drain_inst.ins.sync_info
        if si is not None and si.on_wait and len(si.on_wait) > 1:
            waits = list(si.on_wait)
            drain_inst.ins.sync_info = bass_rust.SyncInfo(
                on_wait=[waits[0]], on_update=list(si.on_update or [])
            )
            for w in waits[1:]:
                n = nc.sync.nop()
                n.ins.sync_info = bass_rust.SyncInfo(on_wait=[w], on_update=[])
        nc.all_engine_barrier()
        assert self.sems is not None
        popped = nc._tile_sem_poison_stack.pop()
        assert popped is self._sem_poison
        nc.clear_and_free_semaphores(list(self.sems.allocated().values()))
        nc.all_engine_barrier()

    tile.TileContext._drain_and_barrier = _drain_and_barrier
    tile.TileContext._drain_patched = True


def _split_excess_waits(nc):
    """Walrus codegen caps sync-waits per instruction (2 for EventSemaphore,
    1 otherwise). Tile's sem assigner can exceed that; move excess waits onto
    single-wait nops inserted just before the instruction on the same engine."""
    import bass_rust
    import concourse.mybir as mybir

    for blk in nc.main_func.blocks:
        il = blk.instructions
        i = 0
        while i < len(il):
            ins = il[i]
            si = ins.sync_info
            if si is not None and si.on_wait:
                cap = 2 if type(ins).__name__ == "InstEventSemaphore" else 1
                if len(si.on_wait) > cap:
                    waits = list(si.on_wait)
                    keep, excess = waits[-cap:], waits[:-cap]
                    ins.sync_info = bass_rust.SyncInfo(
                        on_wait=keep, on_update=list(si.on_update or []))
                    for w in excess:
                        nop = mybir.InstNoOp(
                            name=nc.get_next_instruction_name(), ins=[], outs=[])
                        nop.engine = ins.engine
                        nop.sync_info = bass_rust.SyncInfo(
                            on_wait=[w], on_update=[])
                        nc.register_instruction(nop, overwrite=True)
                        il.insert(i, nop)
                        i += 1
            i += 1


def _build_program(flags):
    import concourse.bass as bass
    import concourse.tile as tile
    from concourse import mybir
    from concourse.masks import make_identity

    _patch_tile_drain()

    F32 = mybir.dt.float32
    F32R = mybir.dt.float32r
    BF16 = mybir.dt.bfloat16
    Act = mybir.ActivationFunctionType
    Alu = mybir.AluOpType
    P, NT, KD, KE = _P, _NT, _KD, _KE
    NH = _N // 2  # seq half

    nc = bass.Bass()
    needed = []

    def din(name, shape):
        needed.append(name)
        return nc.declare_dram_parameter(name, list(shape), F32, isOutput=False)

    xb = din("xb", (_N, _D))
    pos2 = din("pos2", (_N, _D))          # pos_enc + ln0_b
    g0b = None if flags["g0"] else din("g0b", (P, _D))
    wq = din("wq", (P, KD, _D))           # W_q * alpha, [p, ki, f] d=ki*128+p
    wk = din("wk", (P, KD, _D))
    bqc = None if flags["bq"] else din("bqc", (P, KD))
    bkb = None if flags["bk"] else din("bkb", (P, _D))
    m_all = din("m_all", (_H, P, KD, _D))  # M_h = W_v_h @ W_merge_h
    bmb = None if flags["bm"] else din("bmb", (P, _D))
    wff1 = din("wff1", (P, KD, _DE))      # centered diag(ln1_g) @ W_ff1
    bff1c = None if flags["bff1"] else din("bff1c", (P, KE))
    g2c = din("g2c", (P, KE))             # ln2_g col layout
    b2c = din("b2c", (P, KE))             # ln2_b col layout
    wff2 = din("wff2", (P, KE, _D))
    bf2b = None if flags["bf2"] else din("bf2b", (P, _D))
    yout = nc.declare_dram_parameter("y", [_N, _D], F32, isOutput=True)

    xr = xb[:, :].rearrange("(t p) d -> p t d", p=P)
    posr = pos2[:, :].rearrange("(t p) d -> p t d", p=P)
    yr = yout[:, :].rearrange("(t p) d -> p t d", p=P)

    def mm(out, lhsT, rhs, start, stop):
        nc.tensor.matmul(out, lhsT, rhs, start=start, stop=stop)

    with tile.TileContext(nc, pool_alloc_mode="queue") as tc:
        with (
            tc.tile_pool(name="persist", bufs=1) as persist,
            tc.tile_pool(name="pmm", bufs=6, space="PSUM") as pmm,
            tc.tile_pool(name="pstat", bufs=2, space="PSUM") as pstat,
        ):
            warm_f = persist.tile([P, 512], F32)
            nc.vector.memset(warm_f, 1.0)
            warm_t = persist.tile([P, 512], F32R)
            nc.vector.tensor_copy(warm_t[:], warm_f[:])

            x1_t = persist.tile([P, NT, _D], F32)
            ident_f = persist.tile([P, P], F32)
            make_identity(nc, ident_f)
            ident = persist.tile([P, P], F32R)
            nc.vector.tensor_copy(ident[:], ident_f[:])
            ones_f = persist.tile([P, 1], F32)
            nc.vector.memset(ones_f, 1.0)
            ones_bf = persist.tile([P, 1], BF16)
            nc.vector.tensor_copy(ones_bf[:], ones_f[:])
            ones1_f = persist.tile([1, P], F32)
            nc.vector.memset(ones1_f, 1.0)
            ones1_t = persist.tile([1, P], F32R)
            nc.vector.tensor_copy(ones1_t[:], ones1_f[:])
            eps_t = persist.tile([P, 1], F32)
            nc.vector.memset(eps_t, _EPS)

            def filler(n_mm, name):
                """Low-priority PE work the scheduler slots into idle gaps to
                keep the HAM activity monitor at full clock."""
                ps = pstat.tile([P, 512], F32, tag="stat", name=name)
                for w in range(n_mm):
                    mm(ps[:], warm_t[:, :128], warm_t[:],
                       start=(w == 0), stop=(w == n_mm - 1))
                nc.scalar.copy(warm_f[:, 0:1], ps[:, 0:1])

            # A few up-front warm-up matmuls so the first real stream is warm
            filler(16, "warm0")

            # ---------------- Phase A: LN0 + attention + merge ----------------
            with (
                tc.tile_pool(name="phA", bufs=1) as A,
                tc.tile_pool(name="xin", bufs=3) as xin,
                tc.tile_pool(name="posp", bufs=2) as posp,
                tc.tile_pool(name="mstr", bufs=3) as mstr,
                tc.tile_pool(name="xres", bufs=2) as xres,
                tc.tile_pool(name="lnp", bufs=6) as lnp,
            ):
                h_t = A.tile([P, NT, _D], F32R)
                hT_t = A.tile([P, KD, _N], F32R)
                k_t = A.tile([P, NT, _D], F32R)
                qT_t = A.tile([P, KD, _N], F32R)
                sT_t = A.tile([P, KD, _D], F32R)
                wq_t = A.tile([P, KD, _D], F32R)
                wk_t = A.tile([P, KD, _D], F32R)
                if g0b is not None:
                    g0_t = A.tile([P, _D], F32)
                    nc.sync.dma_start(g0_t[:], g0b[:, :])
                if bkb is not None:
                    bk_t = A.tile([P, _D], F32)
                    nc.sync.dma_start(bk_t[:], bkb[:, :])
                if bmb is not None:
                    bm_t = A.tile([P, _D], F32)
                    nc.sync.dma_start(bm_t[:], bmb[:, :])
                if bqc is not None:
                    bq_t = A.tile([P, KD], F32)
                    nc.sync.dma_start(bq_t[:], bqc[:, :])
                sw_ts = [
                    A.tile([P, _D], F32R, tag=f"sw{j}", name=f"sw{j}")
                    for j in range(_H // 2)
                ]

                # LN0 + pos + swish -> h; transpose chunk -> hT
                for t in range(NT):
                    x_c = xin.tile([P, _D], F32, tag="xc", name="xc")
                    nc.sync.dma_start(x_c[:], xr[:, t, :])
                    pos_c = posp.tile([P, _D], F32)
                    nc.sync.dma_start(pos_c[:], posr[:, t, :])
                    st = lnp.tile([P, 6], F32, tag="st")
                    nc.vector.bn_stats(st[:], x_c[:])
                    mv = lnp.tile([P, 2], F32, tag="mv")
                    nc.vector.bn_aggr(mv[:], st[:])
                    rs = lnp.tile([P, 1], F32, tag="rs")
                    nc.scalar.activation(rs[:], mv[:, 1:2], Act.Sqrt,
                                         bias=eps_t[:])
                    nc.vector.reciprocal(rs[:], rs[:])
                    tmp = xin.tile([P, _D], F32, tag="lntmp", name="lntmp")
                    nc.vector.tensor_scalar(
                        tmp[:], x_c[:], mv[:, 0:1], rs[:],
                        op0=Alu.subtract, op1=Alu.mult,
                    )
                    if g0b is not None:
                        nc.vector.tensor_mul(tmp[:], tmp[:], g0_t[:])
                    nc.gpsimd.tensor_add(tmp[:], tmp[:], pos_c[:])
                    nc.scalar.activation(h_t[:, t, :], tmp[:], Act.Silu)
                    pt = pmm.tile([P, 4 * P], F32R, tag="mm", name="ptT")
                    for o in range(KD):
                        nc.tensor.transpose(
                            pt[:, o * P:(o + 1) * P],
                            h_t[:, t, o * P:(o + 1) * P], ident[:]
                        )
                    nc.vector.tensor_copy(
                        hT_t[:, :, t * P:(t + 1) * P],
                        pt[:].rearrange("p (o n) -> p o n", n=P))

                nc.sync.dma_start(wq_t[:], wq[:, :, :].bitcast(F32R))
                nc.sync.dma_start(wk_t[:], wk[:, :, :].bitcast(F32R))

                # qT (feat-major), k (seq-major)
                for fo in range(KD):
                    for s in range(2):
                        pq = pmm.tile([P, 512], F32, tag="mm")
                        for ki in range(KD):
                            mm(pq[:], wq_t[:, ki, fo * P:(fo + 1) * P],
                               hT_t[:, ki, s * 512:(s + 1) * 512],
                               start=(ki == 0), stop=(ki == KD - 1))
                        dst = qT_t[:, fo, s * 512:(s + 1) * 512]
                        if bqc is not None:
                            nc.vector.tensor_scalar_add(dst, pq[:],
                                                        bq_t[:, fo:fo + 1])
                        else:
                            nc.vector.tensor_copy(dst, pq[:])
                for t in range(NT):
                    pk = pmm.tile([P, 512], F32, tag="mm")
                    for ki in range(KD):
                        mm(pk[:], hT_t[:, ki, t * P:(t + 1) * P], wk_t[:, ki, :],
                           start=(ki == 0), stop=(ki == KD - 1))
                    if bkb is not None:
                        nc.vector.tensor_add(k_t[:, t, :], pk[:], bk_t[:])
                    else:
                        nc.scalar.copy(k_t[:, t, :], pk[:])

                # sT = h^T @ k : [d, head*64]
                for o in range(KD):
                    ps = pmm.tile([P, 512], F32, tag="mm")
                    for t in range(NT):
                        mm(ps[:], h_t[:, t, o * P:(o + 1) * P], k_t[:, t, :],
                           start=(t == 0), stop=(t == NT - 1))
                    nc.scalar.copy(sT_t[:, o, :], ps[:])

                # sW_h = s_h @ M_h (64x512); pairs stacked into sw_ts[j]
                # via partition-shifted copyout
                for h_idx in range(_H):
                    mh = mstr.tile([P, KD, _D], F32R)
                    nc.sync.dma_start(mh[:],
                                      m_all[h_idx, :, :, :].bitcast(F32R))
                    pw = pmm.tile([P, 512], F32, tag="mm")
                    for ki in range(KD):
                        mm(pw[:64, :],
                           sT_t[:, ki, h_idx * 64:(h_idx + 1) * 64],
                           mh[:, ki, :],
                           start=(ki == 0), stop=(ki == KD - 1))
                    lo = 64 * (h_idx % 2)
                    nc.scalar.copy(sw_ts[h_idx // 2][lo:lo + 64, :],
                                   pw[:64, :])

                # merged + residual (+ b_merge) -> x1
                for s in range(NT):
                    x_rc = xres.tile([P, _D], F32)
                    nc.sync.dma_start(x_rc[:], xr[:, s, :])
                    pm = pmm.tile([P, 512], F32, tag="mm")
                    for j in range(_H // 2):
                        mm(pm[:], qT_t[:, j, s * P:(s + 1) * P], sw_ts[j][:],
                           start=(j == 0), stop=(j == _H // 2 - 1))
                    x1c = x1_t[:, s, :]
                    nc.vector.tensor_add(x1c, pm[:], x_rc[:])
                    if bmb is not None:
                        nc.vector.tensor_add(x1c, x1c, bm_t[:])


            # ---------------- Phase B: LN1 + FF, two pipelined seq halves ----
            with (
                tc.tile_pool(name="phB", bufs=1) as Bp,
                tc.tile_pool(name="g1T2", bufs=2) as g1Tp,
                tc.tile_pool(name="fT2", bufs=2) as fTp,
                tc.tile_pool(name="rsb2", bufs=2) as rsbp,
                tc.tile_pool(name="row2", bufs=2) as rowp,
                tc.tile_pool(name="g1p", bufs=2) as g1p,
                tc.tile_pool(name="sqp", bufs=3) as sqp,
                tc.tile_pool(name="outp", bufs=2) as outp,
                tc.tile_pool(name="lnp2", bufs=2) as lnp2,
            ):
                wff1_t = Bp.tile([P, KD, _DE], F32R)
                nc.sync.dma_start(wff1_t[:], wff1[:, :, :].bitcast(F32R))
                wff2_t = Bp.tile([P, KE, _D], F32R)
                nc.sync.dma_start(wff2_t[:], wff2[:, :, :].bitcast(F32R))
                g2_t = Bp.tile([P, KE], F32)
                nc.sync.dma_start(g2_t[:], g2c[:, :])
                b2_t = Bp.tile([P, KE], F32)
                nc.sync.dma_start(b2_t[:], b2c[:, :])
                if bff1c is not None:
                    bff1_t = Bp.tile([P, KE], F32)
                    nc.sync.dma_start(bff1_t[:], bff1c[:, :])
                if bf2b is not None:
                    bf2_t = Bp.tile([P, _D], F32)
                    nc.sync.dma_start(bf2_t[:], bf2b[:, :])
                mv1 = Bp.tile([P, NT, 2], F32)
                rs1 = Bp.tile([P, NT], F32)

                # LN1 stats (batched sqrt), apply, transpose — both halves
                for t in range(NT):
                    st = lnp2.tile([P, 6], F32, tag="st")
                    nc.vector.bn_stats(st[:], x1_t[:, t, :])
                    nc.vector.bn_aggr(mv1[:, t, :], st[:])
                nc.scalar.activation(rs1[:], mv1[:, :, 1], Act.Sqrt,
                                     bias=eps_t[:])
                nc.vector.reciprocal(rs1[:], rs1[:])
                g1T_ts = []
                for s in range(2):
                    g1T_t = g1Tp.tile([P, KD, NH], F32R, name=f"g1T{s}")
                    g1T_ts.append(g1T_t)
                    for tt in range(4):
                        t = s * 4 + tt
                        g1c = g1p.tile([P, _D], F32R)
                        nc.vector.tensor_scalar(
                            g1c[:], x1_t[:, t, :], mv1[:, t, 0:1],
                            rs1[:, t:t + 1],
                            op0=Alu.subtract, op1=Alu.mult,
                        )
                        pt = pmm.tile([P, 4 * P], F32R, tag="mm", name="ptG")
                        for o in range(KD):
                            nc.tensor.transpose(
                                pt[:, o * P:(o + 1) * P],
                                g1c[:, o * P:(o + 1) * P], ident[:]
                            )
                        nc.vector.tensor_copy(
                            g1T_t[:, :, tt * P:(tt + 1) * P],
                            pt[:].rearrange("p (o n) -> p o n", n=P))

                for s in range(2):
                    g1T_t = g1T_ts[s]
                    fT_t = fTp.tile([P, KE, NH], F32R)
                    rows = rowp.tile([1, NH], F32R)

                    # fTc = Wff1c^T g1 (pre-centered); fused sumsq stats
                    psq_r = pstat.tile([1, 512], F32, tag="stat")
                    for o in range(KE):
                        pf = pmm.tile([P, 512], F32, tag="mm")
                        for ki in range(KD):
                            mm(pf[:], wff1_t[:, ki, o * P:(o + 1) * P],
                               g1T_t[:, ki, :],
                               start=(ki == 0), stop=(ki == KD - 1))
                        fc = fT_t[:, o, :]
                        if bff1c is not None:
                            nc.vector.tensor_scalar_add(fc, pf[:],
                                                        bff1_t[:, o:o + 1])
                        else:
                            nc.vector.tensor_copy(fc, pf[:])
                        sq = sqp.tile([P, 512], BF16)
                        nc.scalar.activation(sq[:], fc, Act.Square)
                        mm(psq_r[:], ones_bf[:], sq[:],
                           start=(o == 0), stop=(o == KE - 1))

                    # rstd row (scale folded into sqrt) -> broadcast in PSUM
                    with nc.allow_low_precision(
                            reason="f32r rounding of LN2 stats is ~1e-4 rel"):
                        nc.scalar.activation(rows[:, :], psq_r[:], Act.Sqrt,
                                             bias=eps_t[:1, :], scale=1.0 / _DE)
                        nc.vector.reciprocal(rows[:, :], rows[:, :])
                    pb = pmm.tile([P, 512], F32, tag="mm", name="pbb")
                    mm(pb[:], ones1_t[:], rows[:, :], start=True, stop=True)

                    # apply + ff2 fused per o: f2T chunk feeds its ff2
                    # accumulation immediately (no barrier)
                    pos_ = []
                    for tt in range(4):
                        po = pmm.tile([P, 512], F32, tag="mm",
                                      name=f"po{s}_{tt}")
                        pos_.append(po)
                    for o in range(KE):
                        fc = fT_t[:, o, :]
                        nc.vector.tensor_tensor(fc, fc, pb[:], op=Alu.mult)
                        nc.scalar.activation(
                            fc, fc, Act.Silu,
                            bias=b2_t[:, o:o + 1], scale=g2_t[:, o:o + 1],
                        )
                        for tt in range(4):
                            mm(pos_[tt][:], fT_t[:, o, tt * P:(tt + 1) * P],
                               wff2_t[:, o, :],
                               start=(o == 0), stop=(o == KE - 1))

                    # y = x1 + f2T^T @ Wff2 (+ b_ff2)
                    for tt in range(4):
                        t = s * 4 + tt
                        oc = outp.tile([P, _D], F32)
                        nc.vector.tensor_add(oc[:], pos_[tt][:], x1_t[:, t, :])
                        if bf2b is not None:
                            nc.vector.tensor_add(oc[:], oc[:], bf2_t[:])
                        nc.sync.dma_start(yr[:, t, :], oc[:])


    _split_excess_waits(nc)
    return nc, needed


def _host_fold(inputs):
    """Precompute weight layouts/folds. Returns (arrays, flags)."""
    f32 = np.float32
    W_qkv = np.asarray(inputs["W_qkv"], f32)
    b_qkv = np.asarray(inputs["b_qkv"], f32)
    W_merge = np.asarray(inputs["W_merge"], f32)
    alpha = float(np.asarray(inputs["scale"])) ** -0.5

    P = _P

    def col128(w):  # (D, F) -> (128, D//128, F), d = ki*128 + p
        d, f = w.shape
        return np.ascontiguousarray(w.reshape(d // P, P, f).transpose(1, 0, 2))

    def colvec(v):  # (F,) -> (128, F//128), f = o*128 + p
        return np.ascontiguousarray(v.reshape(-1, P).T)

    def bcast(v):  # (D,) -> (128, D)
        return np.ascontiguousarray(np.broadcast_to(v, (P, v.shape[0])))

    Wq = np.ascontiguousarray(W_qkv[:, :_D]) * f32(alpha)
    Wk = np.ascontiguousarray(W_qkv[:, _D:2 * _D])
    bq = b_qkv[:_D] * f32(alpha)
    bk = b_qkv[_D:2 * _D]
    # v-slice bias must be zero for the M_h fold (true for this problem)
    Wv = W_qkv[:, 2 * _D:].reshape(_D, _H, _D)

    M = np.empty((_H, P, _KD, _D), f32)
    Wm64 = W_merge.astype(np.float64).reshape(_H, _D, _D)
    for h in range(_H):
        mh = (Wv[:, h, :].astype(np.float64) @ Wm64[h]).astype(f32)
        M[h] = col128(mh)

    ln0_g = np.asarray(inputs["ln0_g"], f32)
    ln1_g = np.asarray(inputs["ln1_g"], np.float64)
    ln1_b = np.asarray(inputs["ln1_b"], np.float64)
    W_ff1 = np.asarray(inputs["W_ff1"], np.float64)
    w1 = ln1_g[:, None] * W_ff1
    b1 = np.asarray(inputs["b_ff1"], np.float64) + ln1_b @ W_ff1
    # Center so the ff1 matmul emits LN2-pre-centered activations
    w1c = (w1 - w1.mean(axis=1, keepdims=True)).astype(f32)
    b1c = (b1 - b1.mean()).astype(f32)

    b_merge = np.asarray(inputs["b_merge"], f32)
    b_ff2 = np.asarray(inputs["b_ff2"], f32)

    pos2 = (np.asarray(inputs["pos_enc"], f32).reshape(_N, _D)
            + np.asarray(inputs["ln0_b"], f32))

    flags = {
        "g0": bool(np.all(ln0_g == 1.0)),
        "bq": bool(np.all(bq == 0.0)),
        "bk": bool(np.all(bk == 0.0)),
        "bm": bool(np.all(b_merge == 0.0)),
        "bff1": bool(np.all(b1c == 0.0)),
        "bf2": bool(np.all(b_ff2 == 0.0)),
    }

    arrays = {
        "pos2": np.ascontiguousarray(pos2),
        "g0b": bcast(ln0_g),
        "wq": col128(Wq),
        "wk": col128(Wk),
        "bqc": colvec(bq),
        "bkb": bcast(bk),
        "m_all": M,
        "bmb": bcast(b_merge),
        "wff1": col128(w1c),
        "bff1c": colvec(b1c),
        "g2c": colvec(np.asarray(inputs["ln2_g"], f32)),
        "b2c": colvec(np.asarray(inputs["ln2_b"], f32)),
        "wff2": col128(np.asarray(inputs["W_ff2"], f32)),
        "bf2b": bcast(b_ff2),
    }
    return arrays, flags


_PROGRAM_CACHE = {}


def _get_program(flags):
    key = tuple(sorted(flags.items()))
    if key not in _PROGRAM_CACHE:
        _PROGRAM_CACHE[key] = _build_program(flags)
    return _PROGRAM_CACHE[key]


def kernel(**inputs):
    from concourse.bass_utils import run_bass_kernel_spmd

    x = np.asarray(inputs["x"], np.float32)
    arrays, flags = _host_fold(inputs)
    nc, needed = _get_program(flags)

    shared = {k: arrays[k] for k in needed if k != "xb"}
    in_maps = []
    for core in range(_NCORES):
        m = dict(shared)
        m["xb"] = np.ascontiguousarray(x[core])
        in_maps.append(m)

    res = run_bass_kernel_spmd(nc, in_maps, core_ids=list(range(_NCORES)))
    out = np.stack([r["y"] for r in res.results], axis=0)
    return out.astype(np.float32)



# revision 4
# speedup vs baseline: 1.5277x; 1.5277x over previous
"""Trainium2 Bass kernel for nn_DecoderBlock (B=8, N=1024, D=512, H=8, DH=64, DE=2048).

Strategy: 8-way data parallel over batch B — each NeuronCore computes the full
decoder block for one batch element; no collectives.

Algebraic refactors (exact in real arithmetic):
  1. Softmax-free attention is linear. With G := h^T h (symmetric Gram,
     contraction over seq) and M_h := W_v_h @ W_merge_h (host-folded),
         attn_out @ W_merge = h @ T + 1 (x) v,
         T = sum_h Wq_h (Wk_h^T G + bk_h^T r) M_h,   r = 1^T h,
         v = sum_h bq_h (Wk_h^T G + bk_h^T r) M_h,
     collapsing the Q/K projections and the N x N score into D x D
     intermediates (score scale folded into Wq/bq on host).
  2. LN2's mean-centering is folded into W_ff1 (per-row column-mean removed),
     so LN2 only needs a sum-of-squares.

Precision plan (validated off-line vs f64 reference, rel err ~5e-3 against
the 2e-2 budget):
  - attention chain in bf16 operands with f32 PSUM accumulation,
  - ff1/ff2 in fp8e4m3 DoubleRow (2x PE rate): g1 quantized with a x8 scale
    folded into the LN1 rstd, weights x32 on host, descaled on copyout,
  - x / x1 / y residual spine and all LN statistics in f32.

All weight DMAs are issued up-front (bf16/fp8 halves the traffic) so the
ff weights land long before phase B needs them.
"""

import numpy as np

_B, _N, _D = 8, 1024, 512
_H, _DH, _DE = 8, 64, 2048
_EPS = 1e-5
_P = 128
_NT = _N // _P      # 8 seq chunks
_KD = _D // _P      # 4 d chunks
_KE = _DE // _P     # 16 d_expand chunks
_NCORES = 8


def _patch_tile_drain():
    """Walrus in this container caps sync-waits per TPB_CTRL instruction; the
    stock TileContext exit drain attaches one wait per live proc. Split the
    excess onto single-wait SP nops emitted before the semaphore reset."""
    import bass_rust
    import concourse.tile as tile

    if getattr(tile.TileContext, "_drain_patched", False):
        return

    def _drain_and_barrier(self, tick_clock, wait_clock):
        nc = self.nc
        drain_inst = nc.sync.drain()
        wait_clock.add_sem_waits(
            drain_inst.ins, tile.ScopedClock({None: tick_clock.global_clock})
        )
        si = drain_inst.ins.sync_info
        if si is not None and si.on_wait and len(si.on_wait) > 1:
            waits = list(si.on_wait)
            drain_inst.ins.sync_info = bass_rust.SyncInfo(
                on_wait=[waits[0]], on_update=list(si.on_update or [])
            )
            for w in waits[1:]:
                n = nc.sync.nop()
                n.ins.sync_info = bass_rust.SyncInfo(on_wait=[w], on_update=[])
        nc.all_engine_barrier()
        assert self.sems is not None
        popped = nc._tile_sem_poison_stack.pop()
        assert popped is self._sem_poison
        nc.clear_and_free_semaphores(list(self.sems.allocated().values()))
        nc.all_engine_barrier()

    tile.TileContext._drain_and_barrier = _drain_and_barrier
    tile.TileContext._drain_patched = True


def _split_excess_waits(nc):
    """Walrus codegen caps sync-waits per instruction (2 for EventSemaphore,
    1 otherwise). Tile's sem assigner can exceed that; move excess waits onto
    single-wait nops inserted just before the instruction on the same engine."""
    import bass_rust
    import concourse.mybir as mybir

    for blk in nc.main_func.blocks:
        il = blk.instructions
        i = 0
        while i < len(il):
            ins = il[i]
            si = ins.sync_info
            if si is not None and si.on_wait:
                cap = 2 if type(ins).__name__ == "InstEventSemaphore" else 1
                if len(si.on_wait) > cap:
                    waits = list(si.on_wait)
                    keep, excess = waits[-cap:], waits[:-cap]
                    ins.sync_info = bass_rust.SyncInfo(
                        on_wait=keep, on_update=list(si.on_update or []))
                    for w in excess:
                        nop = mybir.InstNoOp(
                            name=nc.get_next_instruction_name(), ins=[], outs=[])
                        nop.engine = ins.engine
                        nop.sync_info = bass_rust.SyncInfo(
                            on_wait=[w], on_update=[])
                        nc.register_instruction(nop, overwrite=True)
                        il.insert(i, nop)
                        i += 1
            i += 1


def _build_program(flags):
    import concourse.bass as bass
    import concourse.tile as tile
    from concourse import mybir
    from concourse.masks import make_identity

    _patch_tile_drain()

    F32 = mybir.dt.float32
    BF16 = mybir.dt.bfloat16
    F8 = mybir.dt.float8e4
    DR = mybir.MatmulPerfMode.DoubleRow
    Act = mybir.ActivationFunctionType
    Alu = mybir.AluOpType
    P, NT, KD, KE = _P, _NT, _KD, _KE
    NH = _N // 2  # seq half
    NPAIR = _H // 2

    nc = bass.Bass()
    needed = []

    def din(name, shape, dt):
        needed.append(name)
        return nc.declare_dram_parameter(name, list(shape), dt, isOutput=False)

    xb = din("xb", (_N, _D), F32)
    pos2 = din("pos2", (_N, _D), BF16)            # pos_enc + ln0_b
    g0b = din("g0b", (P, _D), BF16) if not flags["g0"] else None
    wk = din("wk", (P, KD, _D), BF16)             # (p,c,j) = Wk[c*128+p, j]
    wqp = din("wqp", (P, NPAIR, KD, P), BF16)     # (p,pr,c,i)=Wq_a[c*128+i, pr*128+p]
    m_all = din("m_all", (P, _H, KD, _D), BF16)   # M_h = W_v_h @ W_merge_h
    bkr = din("bkr", (1, _D), BF16) if not flags["bk"] else None
    bqp = din("bqp", (P, NPAIR), BF16) if not flags["bq"] else None
    bmb = din("bmb", (P, _D), F32) if not flags["bm"] else None
    wff1 = din("wff1", (P, KD, _DE), F8)          # centered diag(ln1_g)@W_ff1 x32
    bff1c = din("bff1c", (P, KE), F32) if not flags["bff1"] else None
    g2c = din("g2c", (P, KE), F32) if not flags["g2"] else None
    b2c = din("b2c", (P, KE), F32) if not flags["b2"] else None
    wff2 = din("wff2", (P, KE, _D), F8)           # W_ff2 x32
    bf2b = din("bf2b", (P, _D), F32) if not flags["bf2"] else None
    yout = nc.declare_dram_parameter("y", [_N, _D], F32, isOutput=True)

    xr = xb[:, :].rearrange("(t p) d -> p t d", p=P)
    posr = pos2[:, :].rearrange("(t p) d -> p t d", p=P)
    yr = yout[:, :].rearrange("(t p) d -> p t d", p=P)

    def mm(out, lhsT, rhs, start, stop, **kw):
        nc.tensor.matmul(out, lhsT, rhs, start=start, stop=stop, **kw)

    with tile.TileContext(nc, pool_alloc_mode="queue") as tc:
        with (
            tc.tile_pool(name="persist", bufs=1) as persist,
            tc.tile_pool(name="pmm", bufs=6, space="PSUM") as pmm,
            tc.tile_pool(name="pstat", bufs=2, space="PSUM") as pstat,
            tc.tile_pool(name="lnp", bufs=4) as lnp,
            tc.tile_pool(name="posp", bufs=3) as posp,
            tc.tile_pool(name="sqp", bufs=2) as sqp,
            tc.tile_pool(name="ftp", bufs=3) as ftp,
            tc.tile_pool(name="fc2p", bufs=3) as fc2p,
            tc.tile_pool(name="outp", bufs=3) as outp,
        ):
            # ---- constants --------------------------------------------------
            warm_t = persist.tile([P, 512], BF16)
            nc.vector.memset(warm_t, 0.001)
            ident_f = persist.tile([P, P], F32)
            make_identity(nc, ident_f)
            ident_b = persist.tile([P, P], BF16)
            nc.vector.tensor_copy(ident_b[:], ident_f[:])
            ones_b = persist.tile([P, 1], BF16)
            nc.vector.memset(ones_b, 1.0)
            ones_8 = persist.tile([P, 1], F8)
            nc.vector.memset(ones_8, 1.0)
            ones1_b = persist.tile([1, P], BF16)
            nc.vector.memset(ones1_b, 1.0)
            eps_t = persist.tile([P, 1], F32)
            nc.vector.memset(eps_t, _EPS)
            eps64_t = persist.tile([P, 1], F32)
            nc.vector.memset(eps64_t, _EPS / 64.0)

            def filler(n_mm, name):
                """Low-priority PE chains the scheduler drops into idle slots
                to keep the PE activity monitor (clock) up."""
                ps = pstat.tile([P, 512], F32, tag="stat", name=name)
                for w in range(n_mm):
                    mm(ps[:], warm_t[:, :128], warm_t[:],
                       start=(w == 0), stop=(w == n_mm - 1))
                nc.scalar.copy(warm_t[:, 0:1], ps[:, 0:1])

            filler(12, "warm0")

            # ---- persistent activations/weights -----------------------------
            x_t = persist.tile([P, NT, _D], F32)
            x1_t = persist.tile([P, NT, _D], F32)
            h_t = persist.tile([P, NT, _D], BF16)
            hT_t = persist.tile([P, KD, _N], BF16)
            Gb = persist.tile([P, KD, _D], BF16)
            sTb = persist.tile([P, KD, _D], BF16)
            swb = [persist.tile([P, _D], BF16, name=f"sw{j}")
                   for j in range(NPAIR)]
            Tb = persist.tile([P, KD, _D], BF16)
            wk_t = persist.tile([P, KD, _D], BF16)
            wqp_t = persist.tile([P, NPAIR, KD, P], BF16)
            m_t = persist.tile([P, _H, KD, _D], BF16)
            wff1_t = persist.tile([P, KD, _DE], F8)
            wff2_t = persist.tile([P, KE, _D], F8)
            g1T_t = persist.tile([P, KD, _N], F8)
            fc_t = [persist.tile([P, KE, NH], BF16, name=f"fc{s}")
                    for s in range(2)]
            pb_t = [persist.tile([P, NH], BF16, name=f"pb{s}")
                    for s in range(2)]
            sqv_t = [persist.tile([1, NH], BF16, name=f"sqv{s}")
                     for s in range(2)]
            mv0 = persist.tile([P, NT, 2], F32)
            rs0 = persist.tile([P, NT], F32)
            mv1 = persist.tile([P, NT, 2], F32)
            rs1 = persist.tile([P, NT], F32)

            # ---- input DMAs (weights early; phase B never stalls) -----------
            for t in range(NT):
                nc.sync.dma_start(x_t[:, t, :], xr[:, t, :])
            nc.sync.dma_start(wff1_t[:], wff1[:, :, :])
            nc.sync.dma_start(wff2_t[:], wff2[:, :, :])
            nc.sync.dma_start(wk_t[:], wk[:, :, :])
            nc.sync.dma_start(m_t[:], m_all[:, :, :, :])
            nc.sync.dma_start(wqp_t[:], wqp[:, :, :, :])
            if g0b is not None:
                g0_t = persist.tile([P, _D], BF16)
                nc.sync.dma_start(g0_t[:], g0b[:, :])
            if bkr is not None:
                bkr_t = persist.tile([1, _D], BF16)
                nc.sync.dma_start(bkr_t[:], bkr[:, :])
            if bqp is not None:
                bqp_t = persist.tile([P, NPAIR], BF16)
                nc.sync.dma_start(bqp_t[:], bqp[:, :])
            if bmb is not None:
                bm_t = persist.tile([P, _D], F32)
                nc.sync.dma_start(bm_t[:], bmb[:, :])
            if bff1c is not None:
                bff1_t = persist.tile([P, KE], F32)
                nc.sync.dma_start(bff1_t[:], bff1c[:, :])
            if g2c is not None:
                g2_t = persist.tile([P, KE], F32)
                nc.sync.dma_start(g2_t[:], g2c[:, :])
            if b2c is not None:
                b2_t = persist.tile([P, KE], F32)
                nc.sync.dma_start(b2_t[:], b2c[:, :])
            if bf2b is not None:
                bf2_t = persist.tile([P, _D], F32)
                nc.sync.dma_start(bf2_t[:], bf2b[:, :])

            # ---- LN0 stats (per chunk as x lands), batched rstd -------------
            for t in range(NT):
                st = lnp.tile([P, 6], F32, tag="st")
                nc.vector.bn_stats(st[:], x_t[:, t, :])
                nc.vector.bn_aggr(mv0[:, t, :], st[:])
            nc.scalar.activation(rs0[:], mv0[:, :, 1], Act.Sqrt,
                                 bias=eps_t[:])
            nc.vector.reciprocal(rs0[:], rs0[:])

            # ---- LN0 apply + pos + swish -> h; hT; G accumulation -----------
            gps = [pmm.tile([P, 512], F32, tag="mm", name=f"gacc{c}")
                   for c in range(KD)]
            for t in range(NT):
                pos_c = posp.tile([P, _D], BF16, tag="pos")
                nc.sync.dma_start(pos_c[:], posr[:, t, :])
                tmp = lnp.tile([P, _D], BF16, tag="tmp")
                nc.vector.tensor_scalar(
                    tmp[:], x_t[:, t, :], mv0[:, t, 0:1], rs0[:, t:t + 1],
                    op0=Alu.subtract, op1=Alu.mult,
                )
                if g0b is not None:
                    nc.vector.tensor_mul(tmp[:], tmp[:], g0_t[:])
                tmp2 = lnp.tile([P, _D], BF16, tag="tmp2")
                nc.gpsimd.tensor_add(tmp2[:], tmp[:], pos_c[:])
                nc.scalar.activation(h_t[:, t, :], tmp2[:], Act.Silu)
                pt = pmm.tile([P, KD * P], BF16, tag="mm", name="ptT")
                for c in range(KD):
                    nc.tensor.transpose(
                        pt[:, c * P:(c + 1) * P],
                        h_t[:, t, c * P:(c + 1) * P], ident_b[:])
                nc.vector.tensor_copy(
                    hT_t[:, :, t * P:(t + 1) * P],
                    pt[:].rearrange("p (c n) -> p c n", n=P))
                for c in range(KD):
                    mm(gps[c][:], h_t[:, t, c * P:(c + 1) * P],
                       h_t[:, t, :], start=(t == 0), stop=(t == NT - 1))
            # r = 1^T h (only needed for the bk rank-1 term)
            if bkr is not None:
                rps = pstat.tile([1, 512], F32, tag="stat", name="rrow")
                for t in range(NT):
                    mm(rps[:], ones_b[:], h_t[:, t, :],
                       start=(t == 0), stop=(t == NT - 1))
                r_sb = persist.tile([1, _D], BF16)
                nc.scalar.copy(r_sb[:], rps[:])
            for c in range(KD):
                nc.scalar.copy(Gb[:, c, :], gps[c][:])

            # ---- sT = G @ Wk (+ r^T bk) -------------------------------------
            for c in range(KD):
                ps = pmm.tile([P, 512], F32, tag="mm", name="psT")
                last = KD - 1 if bkr is None else KD
                for k in range(KD):
                    mm(ps[:], Gb[:, k, c * P:(c + 1) * P], wk_t[:, k, :],
                       start=(k == 0), stop=(k == last))
                if bkr is not None:
                    mm(ps[:], r_sb[:, c * P:(c + 1) * P], bkr_t[:],
                       start=False, stop=True)
                nc.scalar.copy(sTb[:, c, :], ps[:])

            # ---- sW_h = s_h @ M_h, pairs stacked ----------------------------
            for h_idx in range(_H):
                pw = pmm.tile([P, 512], F32, tag="mm", name="pw")
                for k in range(KD):
                    mm(pw[:64, :], sTb[:, k, h_idx * 64:(h_idx + 1) * 64],
                       m_t[:, h_idx, k, :],
                       start=(k == 0), stop=(k == KD - 1))
                lo = 64 * (h_idx % 2)
                nc.scalar.copy(swb[h_idx // 2][lo:lo + 64, :], pw[:64, :])

            # ---- T = sum_h Wq_h sW_h ----------------------------------------
            for c in range(KD):
                ps = pmm.tile([P, 512], F32, tag="mm", name="pT")
                for pr in range(NPAIR):
                    mm(ps[:], wqp_t[:, pr, c, :], swb[pr][:],
                       start=(pr == 0), stop=(pr == NPAIR - 1))
                nc.scalar.copy(Tb[:, c, :], ps[:])
            if bqp is not None:
                vps = pstat.tile([1, 512], F32, tag="stat", name="vrow")
                for pr in range(NPAIR):
                    mm(vps[:], bqp_t[:, pr:pr + 1], swb[pr][:],
                       start=(pr == 0), stop=(pr == NPAIR - 1))
                v_sb = persist.tile([1, _D], BF16)
                nc.scalar.copy(v_sb[:], vps[:])

            filler(8, "warmA")

            # ---- merge: x1 = x + h @ T (+ 1 (x) v) (+ b_merge) --------------
            for t in range(NT):
                pm = pmm.tile([P, 512], F32, tag="mm", name="pmg")
                last = KD - 1 if bqp is None else KD
                for c in range(KD):
                    mm(pm[:], hT_t[:, c, t * P:(t + 1) * P], Tb[:, c, :],
                       start=(c == 0), stop=(c == last))
                if bqp is not None:
                    mm(pm[:], ones1_b[:], v_sb[:], start=False, stop=True)
                x1c = x1_t[:, t, :]
                nc.vector.tensor_add(x1c, pm[:], x_t[:, t, :])
                if bmb is not None:
                    nc.vector.tensor_add(x1c, x1c, bm_t[:])

            # ---- LN1 stats + batched rstd (x8 fp8 scale folded) -------------
            for t in range(NT):
                st = lnp.tile([P, 6], F32, tag="st")
                nc.vector.bn_stats(st[:], x1_t[:, t, :])
                nc.vector.bn_aggr(mv1[:, t, :], st[:])
            # rs1 = 8 / sqrt(var + eps) = 1 / sqrt((var + eps)/64)
            nc.scalar.activation(rs1[:], mv1[:, :, 1], Act.Sqrt,
                                 bias=eps64_t[:], scale=1.0 / 64.0)
            nc.vector.reciprocal(rs1[:], rs1[:])

            # ---- g1 (fp8, x8) + transpose -> g1T ----------------------------
            for t in range(NT):
                g1c = lnp.tile([P, _D], BF16, tag="g1c")
                nc.vector.tensor_scalar(
                    g1c[:], x1_t[:, t, :], mv1[:, t, 0:1], rs1[:, t:t + 1],
                    op0=Alu.subtract, op1=Alu.mult,
                )
                pt = pmm.tile([P, KD * P], BF16, tag="mm", name="ptG")
                for c in range(KD):
                    nc.tensor.transpose(
                        pt[:, c * P:(c + 1) * P],
                        g1c[:, c * P:(c + 1) * P], ident_b[:])
                nc.vector.tensor_copy(
                    g1T_t[:, :, t * P:(t + 1) * P],
                    pt[:].rearrange("p (c n) -> p c n", n=P))

            filler(8, "warmB")

            # ---- ff1 both halves: fc = (g1T^T wff1)/256, sq, sumsq ----------
            psq = [pstat.tile([1, NH], F32, tag="stat", name=f"psq{s}")
                   for s in range(2)]
            for s in range(2):
                for o in range(KE):
                    pf = pmm.tile([P, NH], F32, tag="mm", name="pf")
                    for c in range(2):
                        mm(pf[:],
                           wff1_t[:, 2 * c:2 * c + 2, o * P:(o + 1) * P],
                           g1T_t[:, 2 * c:2 * c + 2, s * NH:(s + 1) * NH],
                           start=(c == 0), stop=(c == 1), perf_mode=DR)
                    fc = fc_t[s][:, o, :]
                    if bff1c is not None:
                        nc.vector.tensor_scalar(
                            fc, pf[:], 1.0 / 256.0, bff1_t[:, o:o + 1],
                            op0=Alu.mult, op1=Alu.add)
                    else:
                        nc.scalar.activation(fc, pf[:], Act.Copy,
                                             scale=1.0 / 256.0)
                    sq2 = sqp.tile([P, NH], F8, tag="sq", name="sq2")
                    nc.gpsimd.tensor_tensor(sq2[:], fc, fc, op=Alu.mult)
                    mm(psq[s][:], ones_8[:], sq2[:],
                       start=(o == 0), stop=(o == KE - 1))

            # ---- LN2 rstd, broadcast over partitions via PE -----------------
            for s in range(2):
                nc.scalar.activation(sqv_t[s][:, :], psq[s][:], Act.Sqrt,
                                     bias=eps_t[:1, :], scale=1.0 / _DE)
                pbb = pmm.tile([P, NH], F32, tag="mm", name=f"pbb{s}")
                mm(pbb[:], ones1_b[:], sqv_t[s][:, :], start=True, stop=True)
                with nc.allow_low_precision(
                        reason="bf16 LN2 rstd is ~0.4% — inside budget"):
                    nc.vector.reciprocal(pb_t[s][:], pbb[:])

            filler(6, "warmC")

            # ---- swish(fc * rstd) -> fp8 fT; ff2; y -------------------------
            for s in range(2):
                pos_ = [pmm.tile([P, 512], F32, tag="mm", name=f"po{s}_{tt}")
                        for tt in range(4)]
                for o in range(KE):
                    fc2 = fc2p.tile([P, NH], BF16, tag="fc2")
                    nc.vector.tensor_tensor(fc2[:], fc_t[s][:, o, :],
                                            pb_t[s][:], op=Alu.mult)
                    if o % 2 == 0:
                        fT2 = ftp.tile([P, 2, NH], F8, tag="ft", name="fT2")
                    akw = {}
                    if g2c is not None:
                        akw["scale"] = g2_t[:, o:o + 1]
                    if b2c is not None:
                        akw["bias"] = b2_t[:, o:o + 1]
                    nc.scalar.activation(fT2[:, o % 2, :], fc2[:],
                                         Act.Silu, **akw)
                    if o % 2 == 1:
                        oc = o // 2
                        for tt in range(4):
                            mm(pos_[tt][:],
                               fT2[:, :, tt * P:(tt + 1) * P],
                               wff2_t[:, 2 * oc:2 * oc + 2, :],
                               start=(oc == 0), stop=(oc == KE // 2 - 1),
                               perf_mode=DR)
                # y = x1 + psum/32 (+ b_ff2)
                for tt in range(4):
                    t = s * 4 + tt
                    oc_ = outp.tile([P, _D], F32, tag="oc")
                    nc.vector.scalar_tensor_tensor(
                        oc_[:], pos_[tt][:], 1.0 / 32.0, x1_t[:, t, :],
                        op0=Alu.mult, op1=Alu.add)
                    if bf2b is not None:
                        nc.vector.tensor_add(oc_[:], oc_[:], bf2_t[:])
                    nc.sync.dma_start(yr[:, t, :], oc_[:])

    _split_excess_waits(nc)
    return nc, needed


def _host_fold(inputs):
    """Precompute weight layouts/folds. Returns (arrays, flags)."""
    import ml_dtypes
    f32 = np.float32
    bf16 = ml_dtypes.bfloat16
    f8 = ml_dtypes.float8_e4m3

    W_qkv = np.asarray(inputs["W_qkv"], f32)
    b_qkv = np.asarray(inputs["b_qkv"], f32)
    W_merge = np.asarray(inputs["W_merge"], f32)
    alpha = float(np.asarray(inputs["scale"])) ** -0.5

    P = _P

    def col128(w):  # (D, F) -> (128, D//128, F), d = c*128 + p
        d, f = w.shape
        return np.ascontiguousarray(w.reshape(d // P, P, f).transpose(1, 0, 2))

    def colvec(v):  # (F,) -> (128, F//128), f = o*128 + p
        return np.ascontiguousarray(v.reshape(-1, P).T)

    def bcast(v):  # (D,) -> (128, D)
        return np.ascontiguousarray(np.broadcast_to(v, (P, v.shape[0])))

    Wq = np.ascontiguousarray(W_qkv[:, :_D]) * f32(alpha)
    Wk = np.ascontiguousarray(W_qkv[:, _D:2 * _D])
    bq = b_qkv[:_D] * f32(alpha)
    bk = b_qkv[_D:2 * _D]
    bv = b_qkv[2 * _D:]
    if not np.all(bv == 0.0):
        raise NotImplementedError("nonzero v-bias not supported by the fold")
    Wv = W_qkv[:, 2 * _D:].reshape(_D, _H, _D)

    M = np.empty((P, _H, _KD, _D), bf16)
    Wm64 = W_merge.astype(np.float64).reshape(_H, _D, _D)
    for h in range(_H):
        mh = (Wv[:, h, :].astype(np.float64) @ Wm64[h]).astype(f32)
        M[:, h] = col128(mh).astype(bf16)

    # Wq pairs, transposed: (p, pr, c, i) = Wq[c*128+i, pr*128+p]
    wqp = np.ascontiguousarray(
        Wq.reshape(_KD, P, _H // 2, P).transpose(3, 2, 0, 1)).astype(bf16)

    ln0_g = np.asarray(inputs["ln0_g"], f32)
    ln1_g = np.asarray(inputs["ln1_g"], np.float64)
    ln1_b = np.asarray(inputs["ln1_b"], np.float64)
    W_ff1 = np.asarray(inputs["W_ff1"], np.float64)
    w1 = ln1_g[:, None] * W_ff1
    b1 = np.asarray(inputs["b_ff1"], np.float64) + ln1_b @ W_ff1
    # Center so the ff1 matmul emits LN2-pre-centered activations
    w1c = (w1 - w1.mean(axis=1, keepdims=True)).astype(f32)
    b1c = (b1 - b1.mean()).astype(f32)

    b_merge = np.asarray(inputs["b_merge"], f32)
    b_ff2 = np.asarray(inputs["b_ff2"], f32)
    ln2_g = np.asarray(inputs["ln2_g"], f32)
    ln2_b = np.asarray(inputs["ln2_b"], f32)

    pos2 = (np.asarray(inputs["pos_enc"], f32).reshape(_N, _D)
            + np.asarray(inputs["ln0_b"], f32))

    flags = {
        "g0": bool(np.all(ln0_g == 1.0)),
        "bq": bool(np.all(bq == 0.0)),
        "bk": bool(np.all(bk == 0.0)),
        "bm": bool(np.all(b_merge == 0.0)),
        "bff1": bool(np.all(b1c == 0.0)),
        "g2": bool(np.all(ln2_g == 1.0)),
        "b2": bool(np.all(ln2_b == 0.0)),
        "bf2": bool(np.all(b_ff2 == 0.0)),
    }

    arrays = {
        "pos2": np.ascontiguousarray(pos2).astype(bf16),
        "g0b": bcast(ln0_g).astype(bf16),
        "wk": col128(Wk).astype(bf16),
        "wqp": wqp,
        "m_all": M,
        "bkr": np.ascontiguousarray(bk[None, :]).astype(bf16),
        "bqp": np.ascontiguousarray(bq.reshape(_H // 2, P).T).astype(bf16),
        "bmb": bcast(b_merge),
        "wff1": col128(w1c * 32.0).astype(f8),
        "bff1c": colvec(b1c),
        "g2c": colvec(ln2_g),
        "b2c": colvec(ln2_b),
        "wff2": col128(np.asarray(inputs["W_ff2"], f32) * 32.0).astype(f8),
        "bf2b": bcast(b_ff2),
    }
    return arrays, flags


_PROGRAM_CACHE = {}


def _get_program(flags):
    key = tuple(sorted(flags.items()))
    if key not in _PROGRAM_CACHE:
        _PROGRAM_CACHE[key] = _build_program(flags)
    return _PROGRAM_CACHE[key]


def kernel(**inputs):
    from concourse.bass_utils import run_bass_kernel_spmd

    x = np.asarray(inputs["x"], np.float32)
    arrays, flags = _host_fold(inputs)
    nc, needed = _get_program(flags)

    shared = {k: arrays[k] for k in needed if k != "xb"}
    in_maps = []
    for core in range(_NCORES):
        m = dict(shared)
        m["xb"] = np.ascontiguousarray(x[core])
        in_maps.append(m)

    res = run_bass_kernel_spmd(nc, in_maps, core_ids=list(range(_NCORES)))
    out = np.stack([r["y"] for r in res.results], axis=0)
    return out.astype(np.float32)


# revision 7
# speedup vs baseline: 1.6014x; 1.0483x over previous
"""Trainium2 Bass kernel for nn_DecoderBlock (B=8, N=1024, D=512, H=8, DH=64, DE=2048).

Strategy: 8-way data parallel over batch B — each NeuronCore computes the full
decoder block for one batch element; no collectives.

Algebraic refactors (exact in real arithmetic):
  1. Softmax-free attention is linear. With G := h^T h (symmetric Gram,
     contraction over seq) and M_h := W_v_h @ W_merge_h (host-folded),
         attn_out @ W_merge = h @ T + 1 (x) v,
         T = sum_h Wq_h (Wk_h^T G + bk_h^T r) M_h,   r = 1^T h,
         v = sum_h bq_h (Wk_h^T G + bk_h^T r) M_h,
     collapsing the Q/K projections and the N x N score into D x D
     intermediates (score scale folded into Wq/bq on host).
  2. LN2's mean-centering is folded into W_ff1 (per-row column-mean removed),
     so LN2 only needs a sum-of-squares.

Precision plan (validated off-line vs f64 reference, rel err ~5e-3 against
the 2e-2 budget):
  - attention chain in bf16 operands with f32 PSUM accumulation,
  - ff1/ff2 in fp8e4m3 DoubleRow (2x PE rate): g1 quantized with a x8 scale
    folded into the LN1 rstd, weights x32 on host, descaled on copyout,
  - x / x1 / y residual spine and all LN statistics in f32.

All weight DMAs are issued up-front (bf16/fp8 halves the traffic) so the
ff weights land long before phase B needs them.
"""

import numpy as np

_B, _N, _D = 8, 1024, 512
_H, _DH, _DE = 8, 64, 2048
_EPS = 1e-5
_P = 128
_NT = _N // _P      # 8 seq chunks
_KD = _D // _P      # 4 d chunks
_KE = _DE // _P     # 16 d_expand chunks
_NCORES = 8


def _patch_tile_drain():
    """Walrus in this container caps sync-waits per TPB_CTRL instruction; the
    stock TileContext exit drain attaches one wait per live proc. Split the
    excess onto single-wait SP nops emitted before the semaphore reset."""
    import bass_rust
    import concourse.tile as tile

    if getattr(tile.TileContext, "_drain_patched", False):
        return

    def _drain_and_barrier(self, tick_clock, wait_clock):
        nc = self.nc
        drain_inst = nc.sync.drain()
        wait_clock.add_sem_waits(
            drain_inst.ins, tile.ScopedClock({None: tick_clock.global_clock})
        )
        si = drain_inst.ins.sync_info
        if si is not None and si.on_wait and len(si.on_wait) > 1:
            waits = list(si.on_wait)
            drain_inst.ins.sync_info = bass_rust.SyncInfo(
                on_wait=[waits[0]], on_update=list(si.on_update or [])
            )
            for w in waits[1:]:
                n = nc.sync.nop()
                n.ins.sync_info = bass_rust.SyncInfo(on_wait=[w], on_update=[])
        nc.all_engine_barrier()
        assert self.sems is not None
        popped = nc._tile_sem_poison_stack.pop()
        assert popped is self._sem_poison
        nc.clear_and_free_semaphores(list(self.sems.allocated().values()))
        nc.all_engine_barrier()

    tile.TileContext._drain_and_barrier = _drain_and_barrier
    tile.TileContext._drain_patched = True


def _split_excess_waits(nc):
    """Walrus codegen caps sync-waits per instruction (2 for EventSemaphore,
    1 otherwise). Tile's sem assigner can exceed that; move excess waits onto
    single-wait nops inserted just before the instruction on the same engine."""
    import bass_rust
    import concourse.mybir as mybir

    for blk in nc.main_func.blocks:
        il = blk.instructions
        i = 0
        while i < len(il):
            ins = il[i]
            si = ins.sync_info
            if si is not None and si.on_wait:
                cap = 2 if type(ins).__name__ == "InstEventSemaphore" else 1
                if len(si.on_wait) > cap:
                    waits = list(si.on_wait)
                    keep, excess = waits[-cap:], waits[:-cap]
                    ins.sync_info = bass_rust.SyncInfo(
                        on_wait=keep, on_update=list(si.on_update or []))
                    for w in excess:
                        nop = mybir.InstNoOp(
                            name=nc.get_next_instruction_name(), ins=[], outs=[])
                        nop.engine = ins.engine
                        nop.sync_info = bass_rust.SyncInfo(
                            on_wait=[w], on_update=[])
                        nc.register_instruction(nop, overwrite=True)
                        il.insert(i, nop)
                        i += 1
            i += 1


def _build_program(flags):
    import concourse.bass as bass
    import concourse.tile as tile
    from concourse import mybir
    from concourse.masks import make_identity

    _patch_tile_drain()

    F32 = mybir.dt.float32
    BF16 = mybir.dt.bfloat16
    F8 = mybir.dt.float8e4
    DR = mybir.MatmulPerfMode.DoubleRow
    Act = mybir.ActivationFunctionType
    Alu = mybir.AluOpType
    P, NT, KD, KE = _P, _NT, _KD, _KE
    NH = _N // 2  # seq half
    NPAIR = _H // 2

    nc = bass.Bass()
    needed = []

    def din(name, shape, dt):
        needed.append(name)
        return nc.declare_dram_parameter(name, list(shape), dt, isOutput=False)

    xb = din("xb", (_N, _D), F32)
    xbh = din("xbh", (_N, _D), BF16)              # bf16 x for LN0 (fast arrival)
    pos2 = din("pos2", (_N, _D), BF16)            # pos_enc + ln0_b
    g0b = din("g0b", (P, _D), BF16) if not flags["g0"] else None
    wk = din("wk", (P, KD, _D), BF16)             # (p,c,j) = Wk[c*128+p, j]
    wqp = din("wqp", (P, NPAIR, KD, P), BF16)     # (p,pr,c,i)=Wq_a[c*128+i, pr*128+p]
    m_all = din("m_all", (P, _H, KD, _D), BF16)   # M_h = W_v_h @ W_merge_h
    bkr = din("bkr", (1, _D), BF16) if not flags["bk"] else None
    bqp = din("bqp", (P, NPAIR), BF16) if not flags["bq"] else None
    bmb = din("bmb", (P, _D), F32) if not flags["bm"] else None
    cg = din("cg", (P, KD, _D), BF16)             # C = W1c @ W1c^T (LN2 sumsq)
    wff1 = din("wff1", (P, KD, _DE), F8)          # centered diag(ln1_g)@W_ff1 x32
    bff1c = din("bff1c", (P, KE), F32) if not flags["bff1"] else None
    g2c = din("g2c", (P, KE), F32) if not flags["g2"] else None
    b2c = din("b2c", (P, KE), F32) if not flags["b2"] else None
    wff2 = din("wff2", (P, KE, _D), F8)           # W_ff2 x32
    bf2b = din("bf2b", (P, _D), F32) if not flags["bf2"] else None
    yout = nc.declare_dram_parameter("y", [_N, _D], F32, isOutput=True)

    xr = xb[:, :].rearrange("(t p) d -> p t d", p=P)
    xhr = xbh[:, :].rearrange("(t p) d -> p t d", p=P)
    posr = pos2[:, :].rearrange("(t p) d -> p t d", p=P)
    yr = yout[:, :].rearrange("(t p) d -> p t d", p=P)

    def mm(out, lhsT, rhs, start, stop, **kw):
        nc.tensor.matmul(out, lhsT, rhs, start=start, stop=stop, **kw)

    with tile.TileContext(nc, pool_alloc_mode="queue") as tc:
        with (
            tc.tile_pool(name="persist", bufs=1) as persist,
            tc.tile_pool(name="pmm", bufs=6, space="PSUM") as pmm,
            tc.tile_pool(name="pstat", bufs=2, space="PSUM") as pstat,
            tc.tile_pool(name="lnp", bufs=4) as lnp,
            tc.tile_pool(name="posp", bufs=3) as posp,
            tc.tile_pool(name="sqp", bufs=2) as sqp,
            tc.tile_pool(name="ftp", bufs=3) as ftp,
            tc.tile_pool(name="fc2p", bufs=3) as fc2p,
            tc.tile_pool(name="outp", bufs=3) as outp,
        ):
            # ---- constants --------------------------------------------------
            warm_t = persist.tile([P, 512], BF16)
            nc.vector.memset(warm_t, 0.001)
            ident_f = persist.tile([P, P], F32)
            make_identity(nc, ident_f)
            ident_b = persist.tile([P, P], BF16)
            nc.vector.tensor_copy(ident_b[:], ident_f[:])
            ones_b = persist.tile([P, 1], BF16)
            nc.vector.memset(ones_b, 1.0)
            ones_8 = persist.tile([P, 1], F8)
            nc.vector.memset(ones_8, 1.0)
            ones1_b = persist.tile([1, P], BF16)
            nc.vector.memset(ones1_b, 1.0)
            eps_t = persist.tile([P, 1], F32)
            nc.vector.memset(eps_t, _EPS)
            eps64_t = persist.tile([P, 1], F32)
            nc.vector.memset(eps64_t, _EPS / 64.0)
            eps65536_t = persist.tile([P, 1], F32)
            nc.vector.memset(eps65536_t, _EPS * 65536.0)

            def filler(n_mm, name):
                """Low-priority PE chains the scheduler drops into idle slots
                to keep the PE activity monitor (clock) up."""
                ps = pstat.tile([P, 512], F32, tag="stat", name=name)
                for w in range(n_mm):
                    mm(ps[:], warm_t[:, :128], warm_t[:],
                       start=(w == 0), stop=(w == n_mm - 1))
                nc.scalar.copy(warm_t[:, 0:1], ps[:, 0:1])

            filler(12, "warm0")

            # ---- persistent activations/weights -----------------------------
            x_t = persist.tile([P, NT, _D], F32)
            xh_t = persist.tile([P, NT, _D], BF16)
            x1_t = persist.tile([P, NT, _D], F32)
            h_t = persist.tile([P, NT, _D], BF16)
            hT_t = persist.tile([P, KD, _N], BF16)
            Gb = persist.tile([P, KD, _D], BF16)
            sTb = persist.tile([P, KD, _D], BF16)
            swb = [persist.tile([P, _D], BF16, name=f"sw{j}")
                   for j in range(NPAIR)]
            Tb = persist.tile([P, KD, _D], BF16)
            wk_t = persist.tile([P, KD, _D], BF16)
            wqp_t = persist.tile([P, NPAIR, KD, P], BF16)
            m_t = persist.tile([P, _H, KD, _D], BF16)
            cg_t = persist.tile([P, KD, _D], BF16)
            wff1_t = persist.tile([P, KD, _DE], F8)
            wff2_t = persist.tile([P, KE, _D], F8)
            g1T_t = persist.tile([P, KD, _N], F8)
            pb_t = [persist.tile([P, NH], BF16, name=f"pb{s}")
                    for s in range(2)]
            sqv_t = [persist.tile([1, NH], BF16, name=f"sqv{s}")
                     for s in range(2)]
            mv0 = persist.tile([P, NT, 2], F32)
            rs0 = persist.tile([P, NT], F32)
            nm0 = persist.tile([P, NT], F32)
            nm1 = persist.tile([P, NT], F32)
            mv1 = persist.tile([P, NT, 2], F32)
            rs1 = persist.tile([P, NT], F32)

            # ---- input DMAs (weights early; phase B never stalls) -----------
            for t in range(NT):
                nc.sync.dma_start(xh_t[:, t, :], xhr[:, t, :])
            for t in range(NT):
                nc.sync.dma_start(x_t[:, t, :], xr[:, t, :])
            nc.sync.dma_start(cg_t[:], cg[:, :, :])
            nc.sync.dma_start(wff1_t[:], wff1[:, :, :])
            nc.sync.dma_start(wff2_t[:], wff2[:, :, :])
            nc.sync.dma_start(wk_t[:], wk[:, :, :])
            nc.sync.dma_start(m_t[:], m_all[:, :, :, :])
            nc.sync.dma_start(wqp_t[:], wqp[:, :, :, :])
            if g0b is not None:
                g0_t = persist.tile([P, _D], BF16)
                nc.sync.dma_start(g0_t[:], g0b[:, :])
            if bkr is not None:
                bkr_t = persist.tile([1, _D], BF16)
                nc.sync.dma_start(bkr_t[:], bkr[:, :])
            if bqp is not None:
                bqp_t = persist.tile([P, NPAIR], BF16)
                nc.sync.dma_start(bqp_t[:], bqp[:, :])
            if bmb is not None:
                bm_t = persist.tile([P, _D], F32)
                nc.sync.dma_start(bm_t[:], bmb[:, :])
            if bff1c is not None:
                bff1_t = persist.tile([P, KE], F32)
                nc.sync.dma_start(bff1_t[:], bff1c[:, :])
            if g2c is not None:
                g2_t = persist.tile([P, KE], F32)
                nc.sync.dma_start(g2_t[:], g2c[:, :])
            if b2c is not None:
                b2_t = persist.tile([P, KE], F32)
                nc.sync.dma_start(b2_t[:], b2c[:, :])
            if bf2b is not None:
                bf2_t = persist.tile([P, _D], F32)
                nc.sync.dma_start(bf2_t[:], bf2b[:, :])

            # ---- LN0 stats (per chunk as x lands), batched rstd -------------
            for t in range(NT):
                st = lnp.tile([P, 6], F32, tag="st")
                nc.vector.bn_stats(st[:], xh_t[:, t, :])
                nc.vector.bn_aggr(mv0[:, t, :], st[:])
            nc.scalar.activation(rs0[:], mv0[:, :, 1], Act.Sqrt,
                                 bias=eps_t[:])
            nc.vector.reciprocal(rs0[:], rs0[:])
            nc.vector.scalar_tensor_tensor(nm0[:], mv0[:, :, 0], -1.0,
                                           rs0[:], op0=Alu.mult,
                                           op1=Alu.mult)

            # ---- LN0 apply + pos + swish -> h; hT; G accumulation -----------
            gps = [pmm.tile([P, 512], F32, tag="mm", name=f"gacc{c}")
                   for c in range(KD)]
            for t in range(NT):
                pos_c = posp.tile([P, _D], BF16, tag="pos")
                nc.sync.dma_start(pos_c[:], posr[:, t, :])
                tmp = lnp.tile([P, _D], BF16, tag="tmp")
                if g0b is None:
                    # z1 = x*rstd + pos; -mu*rstd folded into the silu bias
                    nc.vector.scalar_tensor_tensor(
                        tmp[:], xh_t[:, t, :], rs0[:, t:t + 1], pos_c[:],
                        op0=Alu.mult, op1=Alu.add)
                else:
                    nc.vector.tensor_scalar(
                        tmp[:], xh_t[:, t, :], mv0[:, t, 0:1],
                        rs0[:, t:t + 1], op0=Alu.subtract, op1=Alu.mult)
                    nc.vector.tensor_mul(tmp[:], tmp[:], g0_t[:])
                    nc.vector.tensor_add(tmp[:], tmp[:], pos_c[:])
                if g0b is None:
                    nc.scalar.activation(h_t[:, t, :], tmp[:], Act.Silu,
                                         bias=nm0[:, t:t + 1])
                else:
                    nc.scalar.activation(h_t[:, t, :], tmp[:], Act.Silu)
                pt = pmm.tile([P, KD * P], BF16, tag="mm", name="ptT")
                for c in range(KD):
                    nc.tensor.transpose(
                        pt[:, c * P:(c + 1) * P],
                        h_t[:, t, c * P:(c + 1) * P], ident_b[:])
                nc.vector.tensor_copy(
                    hT_t[:, :, t * P:(t + 1) * P],
                    pt[:].rearrange("p (c n) -> p c n", n=P))
                for c in range(KD):
                    mm(gps[c][:], h_t[:, t, c * P:(c + 1) * P],
                       h_t[:, t, :], start=(t == 0), stop=(t == NT - 1))
            # r = 1^T h (only needed for the bk rank-1 term)
            if bkr is not None:
                rps = pstat.tile([1, 512], F32, tag="stat", name="rrow")
                for t in range(NT):
                    mm(rps[:], ones_b[:], h_t[:, t, :],
                       start=(t == 0), stop=(t == NT - 1))
                r_sb = persist.tile([1, _D], BF16)
                nc.scalar.copy(r_sb[:], rps[:])
            for c in range(KD):
                nc.scalar.copy(Gb[:, c, :], gps[c][:])

            # ---- sT = G @ Wk (+ r^T bk) -------------------------------------
            for c in range(KD):
                ps = pmm.tile([P, 512], F32, tag="mm", name="psT")
                last = KD - 1 if bkr is None else KD
                for k in range(KD):
                    mm(ps[:], Gb[:, k, c * P:(c + 1) * P], wk_t[:, k, :],
                       start=(k == 0), stop=(k == last))
                if bkr is not None:
                    mm(ps[:], r_sb[:, c * P:(c + 1) * P], bkr_t[:],
                       start=False, stop=True)
                nc.scalar.copy(sTb[:, c, :], ps[:])

            # ---- sW_h = s_h @ M_h, pairs stacked ----------------------------
            for h_idx in range(_H):
                pw = pmm.tile([P, 512], F32, tag="mm", name="pw")
                for k in range(KD):
                    mm(pw[:64, :], sTb[:, k, h_idx * 64:(h_idx + 1) * 64],
                       m_t[:, h_idx, k, :],
                       start=(k == 0), stop=(k == KD - 1))
                lo = 64 * (h_idx % 2)
                nc.scalar.copy(swb[h_idx // 2][lo:lo + 64, :], pw[:64, :])

            # ---- T = sum_h Wq_h sW_h ----------------------------------------
            for c in range(KD):
                ps = pmm.tile([P, 512], F32, tag="mm", name="pT")
                for pr in range(NPAIR):
                    mm(ps[:], wqp_t[:, pr, c, :], swb[pr][:],
                       start=(pr == 0), stop=(pr == NPAIR - 1))
                nc.scalar.copy(Tb[:, c, :], ps[:])
            if bqp is not None:
                vps = pstat.tile([1, 512], F32, tag="stat", name="vrow")
                for pr in range(NPAIR):
                    mm(vps[:], bqp_t[:, pr:pr + 1], swb[pr][:],
                       start=(pr == 0), stop=(pr == NPAIR - 1))
                v_sb = persist.tile([1, _D], BF16)
                nc.scalar.copy(v_sb[:], vps[:])

            filler(8, "warmA")

            # ---- merge: x1 = x + h @ T (+ 1 (x) v) (+ b_merge) --------------
            for t in range(NT):
                pm = pmm.tile([P, 512], F32, tag="mm", name="pmg")
                last = KD - 1 if bqp is None else KD
                for c in range(KD):
                    mm(pm[:], hT_t[:, c, t * P:(t + 1) * P], Tb[:, c, :],
                       start=(c == 0), stop=(c == last))
                if bqp is not None:
                    mm(pm[:], ones1_b[:], v_sb[:], start=False, stop=True)
                x1c = x1_t[:, t, :]
                nc.vector.tensor_add(x1c, pm[:], x_t[:, t, :])
                if bmb is not None:
                    nc.vector.tensor_add(x1c, x1c, bm_t[:])

            # ---- LN1 stats + batched rstd (x8 fp8 scale folded) -------------
            for t in range(NT):
                st = lnp.tile([P, 6], F32, tag="st")
                nc.vector.bn_stats(st[:], x1_t[:, t, :])
                nc.vector.bn_aggr(mv1[:, t, :], st[:])
            # rs1 = 8 / sqrt(var + eps) = 1 / sqrt((var + eps)/64)
            nc.scalar.activation(rs1[:], mv1[:, :, 1], Act.Sqrt,
                                 bias=eps64_t[:], scale=1.0 / 64.0)
            nc.vector.reciprocal(rs1[:], rs1[:])
            nc.vector.scalar_tensor_tensor(nm1[:], mv1[:, :, 0], -1.0,
                                           rs1[:], op0=Alu.mult,
                                           op1=Alu.mult)

            # ---- g1 (fp8, x8) + transpose -> g1T ----------------------------
            for t in range(NT):
                g1c = lnp.tile([P, _D], BF16, tag="g1c")
                nc.vector.tensor_scalar(
                    g1c[:], x1_t[:, t, :], rs1[:, t:t + 1], nm1[:, t:t + 1],
                    op0=Alu.mult, op1=Alu.add,
                )
                pt = pmm.tile([P, KD * P], BF16, tag="mm", name="ptG")
                for c in range(KD):
                    nc.tensor.transpose(
                        pt[:, c * P:(c + 1) * P],
                        g1c[:, c * P:(c + 1) * P], ident_b[:])
                nc.scalar.copy(
                    g1T_t[:, :, t * P:(t + 1) * P],
                    pt[:].rearrange("p (c n) -> p c n", n=P))

            filler(8, "warmB")

            # ---- LN2 rstd via Gram trick: ssq_n = g1_n^T C g1_n ------------
            # (computable from g1T alone, so rstd is ready before ff1 ends)
            psq = [pstat.tile([1, NH], F32, tag="stat", name=f"psq{s}")
                   for s in range(2)]
            for s in range(2):
                for c in range(KD):
                    zps = pmm.tile([P, NH], F32, tag="mm", name="zps")
                    for k in range(KD):
                        mm(zps[:], cg_t[:, k, c * P:(c + 1) * P],
                           g1T_t[:, k, s * NH:(s + 1) * NH],
                           start=(k == 0), stop=(k == KD - 1))
                    prod = sqp.tile([P, NH], BF16, tag="sq", name="prod")
                    nc.vector.tensor_tensor(
                        prod[:], zps[:],
                        g1T_t[:, c, s * NH:(s + 1) * NH], op=Alu.mult)
                    mm(psq[s][:], ones_b[:], prod[:],
                       start=(c == 0), stop=(c == KD - 1))
                # sqv2 = 256*sqrt(ssq_raw/(64*DE) + eps); pb = rstd2/256
                nc.scalar.activation(sqv_t[s][:, :], psq[s][:], Act.Sqrt,
                                     bias=eps65536_t[:1, :],
                                     scale=65536.0 / (64.0 * _DE))
                pbb = pmm.tile([P, NH], F32, tag="mm", name=f"pbb{s}")
                mm(pbb[:], ones1_b[:], sqv_t[s][:, :], start=True, stop=True)
                with nc.allow_low_precision(
                        reason="bf16 LN2 rstd is ~0.4% — inside budget"):
                    nc.vector.reciprocal(pb_t[s][:], pbb[:])

            # ---- ff1 -> fc2 (straight from PSUM) -> swish -> ff2 -> y -------
            for s in range(2):
                pos_ = [pmm.tile([P, 512], F32, tag="mm", name=f"po{s}_{tt}")
                        for tt in range(4)]
                for o in range(KE):
                    pf = pmm.tile([P, NH], F32, tag="mm", name="pf")
                    for c in range(2):
                        mm(pf[:],
                           wff1_t[:, 2 * c:2 * c + 2, o * P:(o + 1) * P],
                           g1T_t[:, 2 * c:2 * c + 2, s * NH:(s + 1) * NH],
                           start=(c == 0), stop=(c == 1), perf_mode=DR)
                    if o % 2 == 0:
                        fc2 = fc2p.tile([P, 2, NH], BF16, tag="fc2")
                        fT2 = ftp.tile([P, 2, NH], F8, tag="ft", name="fT2")
                    # fc2 = (256*fc) * (rstd2/256) = fc * rstd2
                    nc.vector.tensor_tensor(fc2[:, o % 2, :], pf[:],
                                            pb_t[s][:], op=Alu.mult)
                    if g2c is not None or b2c is not None:
                        # per-o swish so the per-DE-block gain/bias applies
                        akw = {}
                        if g2c is not None:
                            akw["scale"] = g2_t[:, o:o + 1]
                        if b2c is not None:
                            akw["bias"] = b2_t[:, o:o + 1]
                        nc.scalar.activation(fT2[:, o % 2, :],
                                             fc2[:, o % 2, :],
                                             Act.Silu, **akw)
                    elif o % 2 == 1:
                        nc.scalar.activation(
                            fT2[:].rearrange("p a n -> p (a n)"),
                            fc2[:].rearrange("p a n -> p (a n)"),
                            Act.Silu)
                    if o % 2 == 1:
                        oc = o // 2
                        for tt in range(4):
                            mm(pos_[tt][:],
                               fT2[:, :, tt * P:(tt + 1) * P],
                               wff2_t[:, 2 * oc:2 * oc + 2, :],
                               start=(oc == 0), stop=(oc == KE // 2 - 1),
                               perf_mode=DR)
                # y = x1 + psum/32 (+ b_ff2)
                for tt in range(4):
                    t = s * 4 + tt
                    oc_ = outp.tile([P, _D], F32, tag="oc")
                    nc.vector.scalar_tensor_tensor(
                        oc_[:], pos_[tt][:], 1.0 / 32.0, x1_t[:, t, :],
                        op0=Alu.mult, op1=Alu.add)
                    if bf2b is not None:
                        nc.vector.tensor_add(oc_[:], oc_[:], bf2_t[:])
                    nc.sync.dma_start(yr[:, t, :], oc_[:])

    _split_excess_waits(nc)
    return nc, needed


def _host_fold(inputs):
    """Precompute weight layouts/folds. Returns (arrays, flags)."""
    import ml_dtypes
    f32 = np.float32
    bf16 = ml_dtypes.bfloat16
    f8 = ml_dtypes.float8_e4m3

    W_qkv = np.asarray(inputs["W_qkv"], f32)
    b_qkv = np.asarray(inputs["b_qkv"], f32)
    W_merge = np.asarray(inputs["W_merge"], f32)
    alpha = float(np.asarray(inputs["scale"])) ** -0.5

    P = _P

    def col128(w):  # (D, F) -> (128, D//128, F), d = c*128 + p
        d, f = w.shape
        return np.ascontiguousarray(w.reshape(d // P, P, f).transpose(1, 0, 2))

    def colvec(v):  # (F,) -> (128, F//128), f = o*128 + p
        return np.ascontiguousarray(v.reshape(-1, P).T)

    def bcast(v):  # (D,) -> (128, D)
        return np.ascontiguousarray(np.broadcast_to(v, (P, v.shape[0])))

    Wq = np.ascontiguousarray(W_qkv[:, :_D]) * f32(alpha)
    Wk = np.ascontiguousarray(W_qkv[:, _D:2 * _D])
    bq = b_qkv[:_D] * f32(alpha)
    bk = b_qkv[_D:2 * _D]
    bv = b_qkv[2 * _D:]
    if not np.all(bv == 0.0):
        raise NotImplementedError("nonzero v-bias not supported by the fold")
    Wv = W_qkv[:, 2 * _D:].reshape(_D, _H, _D)

    M = np.empty((P, _H, _KD, _D), bf16)
    Wm64 = W_merge.astype(np.float64).reshape(_H, _D, _D)
    for h in range(_H):
        mh = (Wv[:, h, :].astype(np.float64) @ Wm64[h]).astype(f32)
        M[:, h] = col128(mh).astype(bf16)

    # Wq pairs, transposed: (p, pr, c, i) = Wq[c*128+i, pr*128+p]
    wqp = np.ascontiguousarray(
        Wq.reshape(_KD, P, _H // 2, P).transpose(3, 2, 0, 1)).astype(bf16)

    ln0_g = np.asarray(inputs["ln0_g"], f32)
    ln1_g = np.asarray(inputs["ln1_g"], np.float64)
    ln1_b = np.asarray(inputs["ln1_b"], np.float64)
    W_ff1 = np.asarray(inputs["W_ff1"], np.float64)
    w1 = ln1_g[:, None] * W_ff1
    b1 = np.asarray(inputs["b_ff1"], np.float64) + ln1_b @ W_ff1
    # Center so the ff1 matmul emits LN2-pre-centered activations
    w1c = (w1 - w1.mean(axis=1, keepdims=True)).astype(f32)
    b1c = (b1 - b1.mean()).astype(f32)
    if not np.all(b1c == 0.0):
        raise NotImplementedError(
            "nonzero centered ff1 bias breaks the LN2 Gram-trick sumsq")

    b_merge = np.asarray(inputs["b_merge"], f32)
    b_ff2 = np.asarray(inputs["b_ff2"], f32)
    ln2_g = np.asarray(inputs["ln2_g"], f32)
    ln2_b = np.asarray(inputs["ln2_b"], f32)

    pos2 = (np.asarray(inputs["pos_enc"], f32).reshape(_N, _D)
            + np.asarray(inputs["ln0_b"], f32))

    flags = {
        "g0": bool(np.all(ln0_g == 1.0)),
        "bq": bool(np.all(bq == 0.0)),
        "bk": bool(np.all(bk == 0.0)),
        "bm": bool(np.all(b_merge == 0.0)),
        "bff1": bool(np.all(b1c == 0.0)),
        "g2": bool(np.all(ln2_g == 1.0)),
        "b2": bool(np.all(ln2_b == 0.0)),
        "bf2": bool(np.all(b_ff2 == 0.0)),
    }

    arrays = {
        "pos2": np.ascontiguousarray(pos2).astype(bf16),
        "cg": col128((w1c.astype(np.float64) @ w1c.astype(np.float64).T
                      ).astype(f32)).astype(bf16),
        "g0b": bcast(ln0_g).astype(bf16),
        "wk": col128(Wk).astype(bf16),
        "wqp": wqp,
        "m_all": M,
        "bkr": np.ascontiguousarray(bk[None, :]).astype(bf16),
        "bqp": np.ascontiguousarray(bq.reshape(_H // 2, P).T).astype(bf16),
        "bmb": bcast(b_merge),
        "wff1": col128(w1c * 32.0).astype(f8),
        "bff1c": colvec(b1c),
        "g2c": colvec(ln2_g),
        "b2c": colvec(ln2_b),
        "wff2": col128(np.asarray(inputs["W_ff2"], f32) * 32.0).astype(f8),
        "bf2b": bcast(b_ff2),
    }
    return arrays, flags


_PROGRAM_CACHE = {}


def _get_program(flags):
    key = tuple(sorted(flags.items()))
    if key not in _PROGRAM_CACHE:
        _PROGRAM_CACHE[key] = _build_program(flags)
    return _PROGRAM_CACHE[key]


def _in_maps(inputs, arrays, needed):
    import ml_dtypes as _ml
    x = np.asarray(inputs["x"], np.float32)
    shared = {k: arrays[k] for k in needed if k not in ("xb", "xbh")}
    in_maps = []
    for core in range(_NCORES):
        m = dict(shared)
        xc = np.ascontiguousarray(x[core])
        m["xb"] = xc
        if "xbh" in needed:
            m["xbh"] = xc.astype(_ml.bfloat16)
        in_maps.append(m)
    return in_maps


def kernel(**inputs):
    from concourse.bass_utils import run_bass_kernel_spmd

    arrays, flags = _host_fold(inputs)
    nc, needed = _get_program(flags)
    in_maps = _in_maps(inputs, arrays, needed)
    res = run_bass_kernel_spmd(nc, in_maps, core_ids=list(range(_NCORES)))
    out = np.stack([r["y"] for r in res.results], axis=0)
    return out.astype(np.float32)


# revision 9
# speedup vs baseline: 1.9384x; 1.2105x over previous
"""Trainium2 Bass kernel for nn_DecoderBlock (B=8, N=1024, D=512, H=8, DH=64, DE=2048).

Strategy: 8-way data parallel over batch B — each NeuronCore computes the full
decoder block for one batch element; no collectives.

Algebraic refactors (exact in real arithmetic):
  1. Softmax-free attention is linear. With G := h^T h (symmetric Gram,
     contraction over seq) and M_h := W_v_h @ W_merge_h (host-folded),
         attn_out @ W_merge = h @ T + 1 (x) v,
         T = sum_h Wq_h (Wk_h^T G + bk_h^T r) M_h,   r = 1^T h,
         v = sum_h bq_h (Wk_h^T G + bk_h^T r) M_h,
     collapsing the Q/K projections and the N x N score into D x D
     intermediates (score scale folded into Wq/bq on host).
  2. LN2's mean-centering is folded into W_ff1 (per-row column-mean removed),
     so LN2 only needs a sum-of-squares.

Precision plan (validated off-line vs f64 reference, rel err ~5e-3 against
the 2e-2 budget):
  - attention chain in bf16 operands with f32 PSUM accumulation,
  - ff1/ff2 in fp8e4m3 DoubleRow (2x PE rate): g1 quantized with a x8 scale
    folded into the LN1 rstd, weights x32 on host, descaled on copyout,
  - x / x1 / y residual spine and all LN statistics in f32.

All weight DMAs are issued up-front (bf16/fp8 halves the traffic) so the
ff weights land long before phase B needs them.
"""

import numpy as np

_B, _N, _D = 8, 1024, 512
_H, _DH, _DE = 8, 64, 2048
_EPS = 1e-5
_P = 128
_NT = _N // _P      # 8 seq chunks
_KD = _D // _P      # 4 d chunks
_KE = _DE // _P     # 16 d_expand chunks
_NCORES = 8


def _patch_tile_drain():
    """Walrus in this container caps sync-waits per TPB_CTRL instruction; the
    stock TileContext exit drain attaches one wait per live proc. Split the
    excess onto single-wait SP nops emitted before the semaphore reset."""
    import bass_rust
    import concourse.tile as tile

    if getattr(tile.TileContext, "_drain_patched", False):
        return

    def _drain_and_barrier(self, tick_clock, wait_clock):
        nc = self.nc
        drain_inst = nc.sync.drain()
        wait_clock.add_sem_waits(
            drain_inst.ins, tile.ScopedClock({None: tick_clock.global_clock})
        )
        si = drain_inst.ins.sync_info
        if si is not None and si.on_wait and len(si.on_wait) > 1:
            waits = list(si.on_wait)
            drain_inst.ins.sync_info = bass_rust.SyncInfo(
                on_wait=[waits[0]], on_update=list(si.on_update or [])
            )
            for w in waits[1:]:
                n = nc.sync.nop()
                n.ins.sync_info = bass_rust.SyncInfo(on_wait=[w], on_update=[])
        nc.all_engine_barrier()
        assert self.sems is not None
        popped = nc._tile_sem_poison_stack.pop()
        assert popped is self._sem_poison
        nc.clear_and_free_semaphores(list(self.sems.allocated().values()))
        nc.all_engine_barrier()

    tile.TileContext._drain_and_barrier = _drain_and_barrier
    tile.TileContext._drain_patched = True


def _split_excess_waits(nc):
    """Walrus codegen caps sync-waits per instruction (2 for EventSemaphore,
    1 otherwise). Tile's sem assigner can exceed that; move excess waits onto
    single-wait nops inserted just before the instruction on the same engine."""
    import bass_rust
    import concourse.mybir as mybir

    for blk in nc.main_func.blocks:
        il = blk.instructions
        i = 0
        while i < len(il):
            ins = il[i]
            si = ins.sync_info
            if si is not None and si.on_wait:
                cap = 2 if type(ins).__name__ == "InstEventSemaphore" else 1
                if len(si.on_wait) > cap:
                    waits = list(si.on_wait)
                    keep, excess = waits[-cap:], waits[:-cap]
                    ins.sync_info = bass_rust.SyncInfo(
                        on_wait=keep, on_update=list(si.on_update or []))
                    for w in excess:
                        nop = mybir.InstNoOp(
                            name=nc.get_next_instruction_name(), ins=[], outs=[])
                        nop.engine = ins.engine
                        nop.sync_info = bass_rust.SyncInfo(
                            on_wait=[w], on_update=[])
                        nc.register_instruction(nop, overwrite=True)
                        il.insert(i, nop)
                        i += 1
            i += 1


def _build_program(flags):
    import concourse.bass as bass
    import concourse.tile as tile
    from concourse import mybir
    from concourse.masks import make_identity

    _patch_tile_drain()

    F32 = mybir.dt.float32
    BF16 = mybir.dt.bfloat16
    F8 = mybir.dt.float8e4
    DR = mybir.MatmulPerfMode.DoubleRow
    Act = mybir.ActivationFunctionType
    Alu = mybir.AluOpType
    P, NT, KD, KE = _P, _NT, _KD, _KE
    NH = _N // 2  # seq half
    NPAIR = _H // 2

    nc = bass.Bass()
    needed = []

    def din(name, shape, dt):
        needed.append(name)
        return nc.declare_dram_parameter(name, list(shape), dt, isOutput=False)

    xb = din("xb", (_N, _D), F32)
    xbh = din("xbh", (_N, _D), BF16)              # bf16 x for LN0 (fast arrival)
    pos2 = din("pos2", (_N, _D), BF16)            # pos_enc + ln0_b
    g0b = din("g0b", (P, _D), BF16) if not flags["g0"] else None
    wk = din("wk", (P, KD, _D), BF16)             # (p,c,j) = Wk[c*128+p, j]
    wqp = din("wqp", (P, NPAIR, KD, P), BF16)     # (p,pr,c,i)=Wq_a[c*128+i, pr*128+p]
    m_all = din("m_all", (P, _H, KD, _D), BF16)   # M_h = W_v_h @ W_merge_h
    bkr = din("bkr", (1, _D), BF16) if not flags["bk"] else None
    bqp = din("bqp", (P, NPAIR), BF16) if not flags["bq"] else None
    bmb = din("bmb", (P, _D), F32) if not flags["bm"] else None
    cg = din("cg", (P, KD, _D), BF16)             # C = W1c @ W1c^T (LN2 sumsq)
    wff1 = din("wff1", (P, KD, _DE), F8)          # centered diag(ln1_g)@W_ff1 x32
    bff1c = din("bff1c", (P, KE), F32) if not flags["bff1"] else None
    g2c = din("g2c", (P, KE), F32) if not flags["g2"] else None
    b2c = din("b2c", (P, KE), F32) if not flags["b2"] else None
    wff2 = din("wff2", (P, KE, _D), F8)           # W_ff2 x32
    bf2b = din("bf2b", (P, _D), F32) if not flags["bf2"] else None
    yout = nc.declare_dram_parameter("y", [_N, _D], F32, isOutput=True)

    xr = xb[:, :].rearrange("(t p) d -> p t d", p=P)
    xhr = xbh[:, :].rearrange("(t p) d -> p t d", p=P)
    posr = pos2[:, :].rearrange("(t p) d -> p t d", p=P)
    yr = yout[:, :].rearrange("(t p) d -> p t d", p=P)

    def mm(out, lhsT, rhs, start, stop, **kw):
        nc.tensor.matmul(out, lhsT, rhs, start=start, stop=stop, **kw)

    with tile.TileContext(nc, pool_alloc_mode="queue") as tc:
        with (
            tc.tile_pool(name="persist", bufs=1) as persist,
            tc.tile_pool(name="pmm", bufs=6, space="PSUM") as pmm,
            tc.tile_pool(name="pstat", bufs=2, space="PSUM") as pstat,
            tc.tile_pool(name="lnp", bufs=4) as lnp,
            tc.tile_pool(name="sqp", bufs=2) as sqp,
            tc.tile_pool(name="ftp", bufs=3) as ftp,
            tc.tile_pool(name="fc2p", bufs=3) as fc2p,
            tc.tile_pool(name="outp", bufs=3) as outp,
        ):
            # ---- persistent activations/weights -----------------------------
            x_t = persist.tile([P, NT, _D], F32)
            xh_t = persist.tile([P, NT, _D], BF16)
            pos_t = persist.tile([P, NT, _D], BF16)
            x1_t = persist.tile([P, NT, _D], F32)
            h_t = persist.tile([P, NT, _D], BF16)
            hT_t = persist.tile([P, KD, _N], BF16)
            Gb = persist.tile([P, KD, _D], BF16)
            sTb = persist.tile([P, KD, _D], BF16)
            swb = [persist.tile([P, _D], BF16, name=f"sw{j}")
                   for j in range(NPAIR)]
            Tb = persist.tile([P, KD, _D], BF16)
            wk_t = persist.tile([P, KD, _D], BF16)
            wqp_t = persist.tile([P, NPAIR, KD, P], BF16)
            m_t = persist.tile([P, _H, KD, _D], BF16)
            cg_t = persist.tile([P, KD, _D], BF16)
            wff1_t = persist.tile([P, KD, _DE], F8)
            wff2_t = persist.tile([P, KE, _D], F8)
            g1T_t = persist.tile([P, KD, _N], F8)
            pb_t = [persist.tile([P, NH], F32, name=f"pb{s}")
                    for s in range(2)]
            sqv_t = [persist.tile([1, NH], BF16, name=f"sqv{s}")
                     for s in range(2)]
            mv0 = persist.tile([P, NT, 2], F32)
            rs0 = persist.tile([P, NT], F32)
            nm0 = persist.tile([P, NT], F32)
            nm1 = persist.tile([P, NT], F32)
            mv1 = persist.tile([P, NT, 2], F32)
            rs1 = persist.tile([P, NT], F32)

            # ---- input DMAs, in consumption order (xh/pos first) ------------
            for t in range(NT):
                nc.sync.dma_start(xh_t[:, t, :], xhr[:, t, :])
            for t in range(NT):
                nc.sync.dma_start(pos_t[:, t, :], posr[:, t, :])
            nc.sync.dma_start(wk_t[:], wk[:, :, :])
            nc.sync.dma_start(m_t[:], m_all[:, :, :, :])
            nc.sync.dma_start(wqp_t[:], wqp[:, :, :, :])
            for t in range(NT):
                nc.sync.dma_start(x_t[:, t, :], xr[:, t, :])
            nc.sync.dma_start(cg_t[:], cg[:, :, :])
            nc.sync.dma_start(wff1_t[:], wff1[:, :, :])
            nc.sync.dma_start(wff2_t[:], wff2[:, :, :])
            if g0b is not None:
                g0_t = persist.tile([P, _D], BF16)
                nc.sync.dma_start(g0_t[:], g0b[:, :])
            if bkr is not None:
                bkr_t = persist.tile([1, _D], BF16)
                nc.sync.dma_start(bkr_t[:], bkr[:, :])
            if bqp is not None:
                bqp_t = persist.tile([P, NPAIR], BF16)
                nc.sync.dma_start(bqp_t[:], bqp[:, :])
            if bmb is not None:
                bm_t = persist.tile([P, _D], F32)
                nc.sync.dma_start(bm_t[:], bmb[:, :])
            if bff1c is not None:
                bff1_t = persist.tile([P, KE], F32)
                nc.sync.dma_start(bff1_t[:], bff1c[:, :])
            if g2c is not None:
                g2_t = persist.tile([P, KE], F32)
                nc.sync.dma_start(g2_t[:], g2c[:, :])
            if b2c is not None:
                b2_t = persist.tile([P, KE], F32)
                nc.sync.dma_start(b2_t[:], b2c[:, :])
            if bf2b is not None:
                bf2_t = persist.tile([P, _D], F32)
                nc.sync.dma_start(bf2_t[:], bf2b[:, :])

            # ---- constants --------------------------------------------------
            warm_t = persist.tile([P, 512], BF16)
            nc.vector.memset(warm_t, 0.001)
            ident_f = persist.tile([P, P], F32)
            make_identity(nc, ident_f)
            ident_b = persist.tile([P, P], BF16)
            nc.vector.tensor_copy(ident_b[:], ident_f[:])
            ones_b = persist.tile([P, 1], BF16)
            nc.vector.memset(ones_b, 1.0)
            ones_8 = persist.tile([P, 1], F8)
            nc.vector.memset(ones_8, 1.0)
            ones1_b = persist.tile([1, P], BF16)
            nc.vector.memset(ones1_b, 1.0)
            eps_t = persist.tile([P, 1], F32)
            nc.vector.memset(eps_t, _EPS)
            eps64_t = persist.tile([P, 1], F32)
            nc.vector.memset(eps64_t, _EPS / 64.0)
            eps65536_t = persist.tile([P, 1], F32)
            nc.vector.memset(eps65536_t, _EPS * 65536.0)

            def filler(n_mm, name):
                """Low-priority PE chains the scheduler drops into idle slots
                to keep the PE activity monitor (clock) up."""
                ps = pstat.tile([P, 512], F32, tag="stat", name=name)
                for w in range(n_mm):
                    mm(ps[:], warm_t[:, :128], warm_t[:],
                       start=(w == 0), stop=(w == n_mm - 1))
                nc.scalar.copy(warm_t[:, 0:1], ps[:, 0:1])

            filler(12, "warm0")

            # ---- LN0 stats (per chunk as x lands), batched rstd -------------
            for t in range(NT):
                st = lnp.tile([P, 6], F32, tag="st")
                nc.vector.bn_stats(st[:], xh_t[:, t, :])
                nc.vector.bn_aggr(mv0[:, t, :], st[:])
            nc.scalar.activation(rs0[:], mv0[:, :, 1], Act.Sqrt,
                                 bias=eps_t[:])
            nc.vector.reciprocal(rs0[:], rs0[:])
            nc.vector.scalar_tensor_tensor(nm0[:], mv0[:, :, 0], -1.0,
                                           rs0[:], op0=Alu.mult,
                                           op1=Alu.mult)

            # ---- LN0 apply + pos + swish -> h; hT; G accumulation -----------
            gps = [pmm.tile([P, 512], F32, tag="mm", name=f"gacc{c}")
                   for c in range(KD)]
            for t in range(NT):
                tmp = lnp.tile([P, _D], BF16, tag="tmp")
                if g0b is None:
                    # z1 = x*rstd + pos; -mu*rstd folded into the silu bias
                    nc.vector.scalar_tensor_tensor(
                        tmp[:], xh_t[:, t, :], rs0[:, t:t + 1], pos_t[:, t, :],
                        op0=Alu.mult, op1=Alu.add)
                else:
                    nc.vector.tensor_scalar(
                        tmp[:], xh_t[:, t, :], mv0[:, t, 0:1],
                        rs0[:, t:t + 1], op0=Alu.subtract, op1=Alu.mult)
                    nc.vector.tensor_mul(tmp[:], tmp[:], g0_t[:])
                    nc.vector.tensor_add(tmp[:], tmp[:], pos_t[:, t, :])
                if g0b is None:
                    nc.scalar.activation(h_t[:, t, :], tmp[:], Act.Silu,
                                         bias=nm0[:, t:t + 1])
                else:
                    nc.scalar.activation(h_t[:, t, :], tmp[:], Act.Silu)
                pt = pmm.tile([P, KD * P], BF16, tag="mm", name="ptT")
                for c in range(KD):
                    nc.tensor.transpose(
                        pt[:, c * P:(c + 1) * P],
                        h_t[:, t, c * P:(c + 1) * P], ident_b[:])
                nc.vector.tensor_copy(
                    hT_t[:, :, t * P:(t + 1) * P],
                    pt[:].rearrange("p (c n) -> p c n", n=P))
                for c in range(KD):
                    mm(gps[c][:], h_t[:, t, c * P:(c + 1) * P],
                       h_t[:, t, :], start=(t == 0), stop=(t == NT - 1))
            # r = 1^T h (only needed for the bk rank-1 term)
            if bkr is not None:
                rps = pstat.tile([1, 512], F32, tag="stat", name="rrow")
                for t in range(NT):
                    mm(rps[:], ones_b[:], h_t[:, t, :],
                       start=(t == 0), stop=(t == NT - 1))
                r_sb = persist.tile([1, _D], BF16)
                nc.scalar.copy(r_sb[:], rps[:])
            for c in range(KD):
                nc.scalar.copy(Gb[:, c, :], gps[c][:])

            # ---- sT = G @ Wk (+ r^T bk) -------------------------------------
            for c in range(KD):
                ps = pmm.tile([P, 512], F32, tag="mm", name="psT")
                last = KD - 1 if bkr is None else KD
                for k in range(KD):
                    mm(ps[:], Gb[:, k, c * P:(c + 1) * P], wk_t[:, k, :],
                       start=(k == 0), stop=(k == last))
                if bkr is not None:
                    mm(ps[:], r_sb[:, c * P:(c + 1) * P], bkr_t[:],
                       start=False, stop=True)
                nc.scalar.copy(sTb[:, c, :], ps[:])

            # ---- sW_h = s_h @ M_h, pairs stacked ----------------------------
            for h_idx in range(_H):
                pw = pmm.tile([P, 512], F32, tag="mm", name="pw")
                for k in range(KD):
                    mm(pw[:64, :], sTb[:, k, h_idx * 64:(h_idx + 1) * 64],
                       m_t[:, h_idx, k, :],
                       start=(k == 0), stop=(k == KD - 1))
                lo = 64 * (h_idx % 2)
                nc.scalar.copy(swb[h_idx // 2][lo:lo + 64, :], pw[:64, :])

            # ---- T = sum_h Wq_h sW_h ----------------------------------------
            for c in range(KD):
                ps = pmm.tile([P, 512], F32, tag="mm", name="pT")
                for pr in range(NPAIR):
                    mm(ps[:], wqp_t[:, pr, c, :], swb[pr][:],
                       start=(pr == 0), stop=(pr == NPAIR - 1))
                nc.scalar.copy(Tb[:, c, :], ps[:])
            if bqp is not None:
                vps = pstat.tile([1, 512], F32, tag="stat", name="vrow")
                for pr in range(NPAIR):
                    mm(vps[:], bqp_t[:, pr:pr + 1], swb[pr][:],
                       start=(pr == 0), stop=(pr == NPAIR - 1))
                v_sb = persist.tile([1, _D], BF16)
                nc.scalar.copy(v_sb[:], vps[:])

            filler(8, "warmA")

            # ---- merge: x1 = x + h @ T (+ 1 (x) v) (+ b_merge) --------------
            for t in range(NT):
                pm = pmm.tile([P, 512], F32, tag="mm", name="pmg")
                last = KD - 1 if bqp is None else KD
                for c in range(KD):
                    mm(pm[:], hT_t[:, c, t * P:(t + 1) * P], Tb[:, c, :],
                       start=(c == 0), stop=(c == last))
                if bqp is not None:
                    mm(pm[:], ones1_b[:], v_sb[:], start=False, stop=True)
                x1c = x1_t[:, t, :]
                nc.vector.tensor_add(x1c, pm[:], x_t[:, t, :])
                if bmb is not None:
                    nc.vector.tensor_add(x1c, x1c, bm_t[:])

            # ---- LN1 stats + batched rstd (x8 fp8 scale folded) -------------
            for t in range(NT):
                st = lnp.tile([P, 6], F32, tag="st")
                nc.vector.bn_stats(st[:], x1_t[:, t, :])
                nc.vector.bn_aggr(mv1[:, t, :], st[:])
            # rs1 = 8 / sqrt(var + eps) = 1 / sqrt((var + eps)/64)
            nc.scalar.activation(rs1[:], mv1[:, :, 1], Act.Sqrt,
                                 bias=eps64_t[:], scale=1.0 / 64.0)
            nc.vector.reciprocal(rs1[:], rs1[:])
            nc.vector.scalar_tensor_tensor(nm1[:], mv1[:, :, 0], -1.0,
                                           rs1[:], op0=Alu.mult,
                                           op1=Alu.mult)

            # ---- g1 (fp8, x8) + transpose -> g1T ----------------------------
            for t in range(NT):
                g1c = lnp.tile([P, _D], BF16, tag="g1c")
                nc.vector.tensor_scalar(
                    g1c[:], x1_t[:, t, :], rs1[:, t:t + 1], nm1[:, t:t + 1],
                    op0=Alu.mult, op1=Alu.add,
                )
                pt = pmm.tile([P, KD * P], BF16, tag="mm", name="ptG")
                for c in range(KD):
                    nc.tensor.transpose(
                        pt[:, c * P:(c + 1) * P],
                        g1c[:, c * P:(c + 1) * P], ident_b[:])
                nc.scalar.copy(
                    g1T_t[:, :, t * P:(t + 1) * P],
                    pt[:].rearrange("p (c n) -> p c n", n=P))

            filler(8, "warmB")

            # ---- LN2 rstd via Gram trick: ssq_n = g1_n^T C g1_n ------------
            # (computable from g1T alone, so rstd is ready before ff1 ends)
            psq = [pstat.tile([1, NH], F32, tag="stat", name=f"psq{s}")
                   for s in range(2)]
            for s in range(2):
                for c in range(KD):
                    zps = pmm.tile([P, NH], F32, tag="mm", name="zps")
                    for k in range(KD):
                        mm(zps[:], cg_t[:, k, c * P:(c + 1) * P],
                           g1T_t[:, k, s * NH:(s + 1) * NH],
                           start=(k == 0), stop=(k == KD - 1))
                    prod = sqp.tile([P, NH], BF16, tag="sq", name="prod")
                    nc.vector.tensor_tensor(
                        prod[:], zps[:],
                        g1T_t[:, c, s * NH:(s + 1) * NH], op=Alu.mult)
                    mm(psq[s][:], ones_b[:], prod[:],
                       start=(c == 0), stop=(c == KD - 1))

            # ---- ff1 -> fc2 (straight from PSUM) -> swish -> ff2 -> y -------
            for s in range(2):
                # finalize this half's rstd just-in-time (keeps the scalar
                # queue free of cross-half stat ops ahead of the swishes)
                # sqv2 = 256*sqrt(ssq_raw/(64*DE) + eps); pb = rstd2/256
                nc.scalar.activation(sqv_t[s][:, :], psq[s][:], Act.Sqrt,
                                     bias=eps65536_t[:1, :],
                                     scale=65536.0 / (64.0 * _DE))
                pbb = pmm.tile([P, NH], F32, tag="mm", name=f"pbb{s}")
                mm(pbb[:], ones1_b[:], sqv_t[s][:, :], start=True, stop=True)
                nc.vector.reciprocal(pb_t[s][:], pbb[:])
                pos_ = [pmm.tile([P, 512], F32, tag="mm", name=f"po{s}_{tt}")
                        for tt in range(4)]
                for o in range(KE):
                    pf = pmm.tile([P, NH], F32, tag="mm", name="pf")
                    for c in range(2):
                        mm(pf[:],
                           wff1_t[:, 2 * c:2 * c + 2, o * P:(o + 1) * P],
                           g1T_t[:, 2 * c:2 * c + 2, s * NH:(s + 1) * NH],
                           start=(c == 0), stop=(c == 1), perf_mode=DR)
                    if o % 2 == 0:
                        fc2 = fc2p.tile([P, 2, NH], BF16, tag="fc2")
                        fT2 = ftp.tile([P, 2, NH], F8, tag="ft", name="fT2")
                    # fc2 = (256*fc) * (rstd2/256) = fc * rstd2
                    nc.vector.tensor_tensor(fc2[:, o % 2, :], pf[:],
                                            pb_t[s][:], op=Alu.mult)
                    if g2c is not None or b2c is not None:
                        # per-o swish so the per-DE-block gain/bias applies
                        akw = {}
                        if g2c is not None:
                            akw["scale"] = g2_t[:, o:o + 1]
                        if b2c is not None:
                            akw["bias"] = b2_t[:, o:o + 1]
                        nc.scalar.activation(fT2[:, o % 2, :],
                                             fc2[:, o % 2, :],
                                             Act.Silu, **akw)
                    elif o % 2 == 1:
                        nc.scalar.activation(
                            fT2[:].rearrange("p a n -> p (a n)"),
                            fc2[:].rearrange("p a n -> p (a n)"),
                            Act.Silu)
                    if o % 2 == 1:
                        oc = o // 2
                        for tt in range(4):
                            mm(pos_[tt][:],
                               fT2[:, :, tt * P:(tt + 1) * P],
                               wff2_t[:, 2 * oc:2 * oc + 2, :],
                               start=(oc == 0), stop=(oc == KE // 2 - 1),
                               perf_mode=DR)
                # y = x1 + psum/32 (+ b_ff2)
                for tt in range(4):
                    t = s * 4 + tt
                    oc_ = outp.tile([P, _D], F32, tag="oc")
                    nc.vector.scalar_tensor_tensor(
                        oc_[:], pos_[tt][:], 1.0 / 32.0, x1_t[:, t, :],
                        op0=Alu.mult, op1=Alu.add)
                    if bf2b is not None:
                        nc.vector.tensor_add(oc_[:], oc_[:], bf2_t[:])
                    nc.sync.dma_start(yr[:, t, :], oc_[:])

    _split_excess_waits(nc)
    return nc, needed


def _host_fold(inputs):
    """Precompute weight layouts/folds. Returns (arrays, flags)."""
    import ml_dtypes
    f32 = np.float32
    bf16 = ml_dtypes.bfloat16
    f8 = ml_dtypes.float8_e4m3

    W_qkv = np.asarray(inputs["W_qkv"], f32)
    b_qkv = np.asarray(inputs["b_qkv"], f32)
    W_merge = np.asarray(inputs["W_merge"], f32)
    alpha = float(np.asarray(inputs["scale"])) ** -0.5

    P = _P

    def col128(w):  # (D, F) -> (128, D//128, F), d = c*128 + p
        d, f = w.shape
        return np.ascontiguousarray(w.reshape(d // P, P, f).transpose(1, 0, 2))

    def colvec(v):  # (F,) -> (128, F//128), f = o*128 + p
        return np.ascontiguousarray(v.reshape(-1, P).T)

    def bcast(v):  # (D,) -> (128, D)
        return np.ascontiguousarray(np.broadcast_to(v, (P, v.shape[0])))

    Wq = np.ascontiguousarray(W_qkv[:, :_D]) * f32(alpha)
    Wk = np.ascontiguousarray(W_qkv[:, _D:2 * _D])
    bq = b_qkv[:_D] * f32(alpha)
    bk = b_qkv[_D:2 * _D]
    bv = b_qkv[2 * _D:]
    if not np.all(bv == 0.0):
        raise NotImplementedError("nonzero v-bias not supported by the fold")
    Wv = W_qkv[:, 2 * _D:].reshape(_D, _H, _D)

    M = np.empty((P, _H, _KD, _D), bf16)
    Wm64 = W_merge.astype(np.float64).reshape(_H, _D, _D)
    for h in range(_H):
        mh = (Wv[:, h, :].astype(np.float64) @ Wm64[h]).astype(f32)
        M[:, h] = col128(mh).astype(bf16)

    # Wq pairs, transposed: (p, pr, c, i) = Wq[c*128+i, pr*128+p]
    wqp = np.ascontiguousarray(
        Wq.reshape(_KD, P, _H // 2, P).transpose(3, 2, 0, 1)).astype(bf16)

    ln0_g = np.asarray(inputs["ln0_g"], f32)
    ln1_g = np.asarray(inputs["ln1_g"], np.float64)
    ln1_b = np.asarray(inputs["ln1_b"], np.float64)
    W_ff1 = np.asarray(inputs["W_ff1"], np.float64)
    w1 = ln1_g[:, None] * W_ff1
    b1 = np.asarray(inputs["b_ff1"], np.float64) + ln1_b @ W_ff1
    # Center so the ff1 matmul emits LN2-pre-centered activations
    w1c = (w1 - w1.mean(axis=1, keepdims=True)).astype(f32)
    b1c = (b1 - b1.mean()).astype(f32)
    if not np.all(b1c == 0.0):
        raise NotImplementedError(
            "nonzero centered ff1 bias breaks the LN2 Gram-trick sumsq")

    b_merge = np.asarray(inputs["b_merge"], f32)
    b_ff2 = np.asarray(inputs["b_ff2"], f32)
    ln2_g = np.asarray(inputs["ln2_g"], f32)
    ln2_b = np.asarray(inputs["ln2_b"], f32)

    pos2 = (np.asarray(inputs["pos_enc"], f32).reshape(_N, _D)
            + np.asarray(inputs["ln0_b"], f32))

    flags = {
        "g0": bool(np.all(ln0_g == 1.0)),
        "bq": bool(np.all(bq == 0.0)),
        "bk": bool(np.all(bk == 0.0)),
        "bm": bool(np.all(b_merge == 0.0)),
        "bff1": bool(np.all(b1c == 0.0)),
        "g2": bool(np.all(ln2_g == 1.0)),
        "b2": bool(np.all(ln2_b == 0.0)),
        "bf2": bool(np.all(b_ff2 == 0.0)),
    }

    arrays = {
        "pos2": np.ascontiguousarray(pos2).astype(bf16),
        "cg": col128((w1c.astype(np.float64) @ w1c.astype(np.float64).T
                      ).astype(f32)).astype(bf16),
        "g0b": bcast(ln0_g).astype(bf16),
        "wk": col128(Wk).astype(bf16),
        "wqp": wqp,
        "m_all": M,
        "bkr": np.ascontiguousarray(bk[None, :]).astype(bf16),
        "bqp": np.ascontiguousarray(bq.reshape(_H // 2, P).T).astype(bf16),
        "bmb": bcast(b_merge),
        "wff1": col128(w1c * 32.0).astype(f8),
        "bff1c": colvec(b1c),
        "g2c": colvec(ln2_g),
        "b2c": colvec(ln2_b),
        "wff2": col128(np.asarray(inputs["W_ff2"], f32) * 32.0).astype(f8),
        "bf2b": bcast(b_ff2),
    }
    return arrays, flags


_PROGRAM_CACHE = {}


def _get_program(flags):
    key = tuple(sorted(flags.items()))
    if key not in _PROGRAM_CACHE:
        _PROGRAM_CACHE[key] = _build_program(flags)
    return _PROGRAM_CACHE[key]


def _in_maps(inputs, arrays, needed):
    import ml_dtypes as _ml
    x = np.asarray(inputs["x"], np.float32)
    shared = {k: arrays[k] for k in needed if k not in ("xb", "xbh")}
    in_maps = []
    for core in range(_NCORES):
        m = dict(shared)
        xc = np.ascontiguousarray(x[core])
        m["xb"] = xc
        if "xbh" in needed:
            m["xbh"] = xc.astype(_ml.bfloat16)
        in_maps.append(m)
    return in_maps


def kernel(**inputs):
    from concourse.bass_utils import run_bass_kernel_spmd

    arrays, flags = _host_fold(inputs)
    nc, needed = _get_program(flags)
    in_maps = _in_maps(inputs, arrays, needed)
    res = run_bass_kernel_spmd(nc, in_maps, core_ids=list(range(_NCORES)))
    out = np.stack([r["y"] for r in res.results], axis=0)
    return out.astype(np.float32)


# revision 10
# speedup vs baseline: 1.9477x; 1.0048x over previous
"""Trainium2 Bass kernel for nn_DecoderBlock (B=8, N=1024, D=512, H=8, DH=64, DE=2048).

Strategy: 8-way data parallel over batch B — each NeuronCore computes the full
decoder block for one batch element; no collectives.

Algebraic refactors (exact in real arithmetic):
  1. Softmax-free attention is linear. With G := h^T h (symmetric Gram,
     contraction over seq) and M_h := W_v_h @ W_merge_h (host-folded),
         attn_out @ W_merge = h @ T + 1 (x) v,
         T = sum_h Wq_h (Wk_h^T G + bk_h^T r) M_h,   r = 1^T h,
         v = sum_h bq_h (Wk_h^T G + bk_h^T r) M_h,
     collapsing the Q/K projections and the N x N score into D x D
     intermediates (score scale folded into Wq/bq on host).
  2. LN2's mean-centering is folded into W_ff1 (per-row column-mean removed),
     so LN2 only needs a sum-of-squares.

Precision plan (validated off-line vs f64 reference, rel err ~5e-3 against
the 2e-2 budget):
  - attention chain in bf16 operands with f32 PSUM accumulation,
  - ff1/ff2 in fp8e4m3 DoubleRow (2x PE rate): g1 quantized with a x8 scale
    folded into the LN1 rstd, weights x32 on host, descaled on copyout,
  - x / x1 / y residual spine and all LN statistics in f32.

All weight DMAs are issued up-front (bf16/fp8 halves the traffic) so the
ff weights land long before phase B needs them.
"""

import numpy as np

_B, _N, _D = 8, 1024, 512
_H, _DH, _DE = 8, 64, 2048
_EPS = 1e-5
_P = 128
_NT = _N // _P      # 8 seq chunks
_KD = _D // _P      # 4 d chunks
_KE = _DE // _P     # 16 d_expand chunks
_NCORES = 8


def _patch_tile_drain():
    """Walrus in this container caps sync-waits per TPB_CTRL instruction; the
    stock TileContext exit drain attaches one wait per live proc. Split the
    excess onto single-wait SP nops emitted before the semaphore reset."""
    import bass_rust
    import concourse.tile as tile

    if getattr(tile.TileContext, "_drain_patched", False):
        return

    def _drain_and_barrier(self, tick_clock, wait_clock):
        nc = self.nc
        drain_inst = nc.sync.drain()
        wait_clock.add_sem_waits(
            drain_inst.ins, tile.ScopedClock({None: tick_clock.global_clock})
        )
        si = drain_inst.ins.sync_info
        if si is not None and si.on_wait and len(si.on_wait) > 1:
            waits = list(si.on_wait)
            drain_inst.ins.sync_info = bass_rust.SyncInfo(
                on_wait=[waits[0]], on_update=list(si.on_update or [])
            )
            for w in waits[1:]:
                n = nc.sync.nop()
                n.ins.sync_info = bass_rust.SyncInfo(on_wait=[w], on_update=[])
        nc.all_engine_barrier()
        assert self.sems is not None
        popped = nc._tile_sem_poison_stack.pop()
        assert popped is self._sem_poison
        nc.clear_and_free_semaphores(list(self.sems.allocated().values()))
        nc.all_engine_barrier()

    tile.TileContext._drain_and_barrier = _drain_and_barrier
    tile.TileContext._drain_patched = True


def _split_excess_waits(nc):
    """Walrus codegen caps sync-waits per instruction (2 for EventSemaphore,
    1 otherwise). Tile's sem assigner can exceed that; move excess waits onto
    single-wait nops inserted just before the instruction on the same engine."""
    import bass_rust
    import concourse.mybir as mybir

    for blk in nc.main_func.blocks:
        il = blk.instructions
        i = 0
        while i < len(il):
            ins = il[i]
            si = ins.sync_info
            if si is not None and si.on_wait:
                cap = 2 if type(ins).__name__ == "InstEventSemaphore" else 1
                if len(si.on_wait) > cap:
                    waits = list(si.on_wait)
                    keep, excess = waits[-cap:], waits[:-cap]
                    ins.sync_info = bass_rust.SyncInfo(
                        on_wait=keep, on_update=list(si.on_update or []))
                    for w in excess:
                        nop = mybir.InstNoOp(
                            name=nc.get_next_instruction_name(), ins=[], outs=[])
                        nop.engine = ins.engine
                        nop.sync_info = bass_rust.SyncInfo(
                            on_wait=[w], on_update=[])
                        nc.register_instruction(nop, overwrite=True)
                        il.insert(i, nop)
                        i += 1
            i += 1


def _build_program(flags):
    import concourse.bass as bass
    import concourse.tile as tile
    from concourse import mybir
    from concourse.masks import make_identity

    _patch_tile_drain()

    F32 = mybir.dt.float32
    BF16 = mybir.dt.bfloat16
    F8 = mybir.dt.float8e4
    DR = mybir.MatmulPerfMode.DoubleRow
    Act = mybir.ActivationFunctionType
    Alu = mybir.AluOpType
    P, NT, KD, KE = _P, _NT, _KD, _KE
    NH = _N // 2  # seq half
    NPAIR = _H // 2

    nc = bass.Bass()
    needed = []

    def din(name, shape, dt):
        needed.append(name)
        return nc.declare_dram_parameter(name, list(shape), dt, isOutput=False)

    xb = din("xb", (_N, _D), F32)
    xbh = din("xbh", (_N, _D), BF16)              # bf16 x for LN0 (fast arrival)
    pos2 = din("pos2", (_N, _D), BF16)            # pos_enc + ln0_b
    g0b = din("g0b", (P, _D), BF16) if not flags["g0"] else None
    wk = din("wk", (P, KD, _D), BF16)             # (p,c,j) = Wk[c*128+p, j]
    wqp = din("wqp", (P, NPAIR, KD, P), BF16)     # (p,pr,c,i)=Wq_a[c*128+i, pr*128+p]
    m_all = din("m_all", (P, _H, KD, _D), BF16)   # M_h = W_v_h @ W_merge_h
    bkr = din("bkr", (1, _D), BF16) if not flags["bk"] else None
    bqp = din("bqp", (P, NPAIR), BF16) if not flags["bq"] else None
    bmb = din("bmb", (P, _D), F32) if not flags["bm"] else None
    cg = din("cg", (P, KD, _D), BF16)             # C = W1c @ W1c^T (LN2 sumsq)
    wff1 = din("wff1", (P, KD, _DE), F8)          # centered diag(ln1_g)@W_ff1 x32
    bff1c = din("bff1c", (P, KE), F32) if not flags["bff1"] else None
    g2c = din("g2c", (P, KE), F32) if not flags["g2"] else None
    b2c = din("b2c", (P, KE), F32) if not flags["b2"] else None
    wff2 = din("wff2", (P, KE, _D), F8)           # W_ff2 x32
    bf2b = din("bf2b", (P, _D), F32) if not flags["bf2"] else None
    yout = nc.declare_dram_parameter("y", [_N, _D], F32, isOutput=True)

    xr = xb[:, :].rearrange("(t p) d -> p t d", p=P)
    xhr = xbh[:, :].rearrange("(t p) d -> p t d", p=P)
    posr = pos2[:, :].rearrange("(t p) d -> p t d", p=P)
    yr = yout[:, :].rearrange("(t p) d -> p t d", p=P)

    def mm(out, lhsT, rhs, start, stop, **kw):
        nc.tensor.matmul(out, lhsT, rhs, start=start, stop=stop, **kw)

    with tile.TileContext(nc, pool_alloc_mode="queue") as tc:
        with (
            tc.tile_pool(name="persist", bufs=1) as persist,
            tc.tile_pool(name="pmm", bufs=4, space="PSUM") as pmm,
            tc.tile_pool(name="lnp", bufs=4) as lnp,
            tc.tile_pool(name="sqp", bufs=2) as sqp,
            tc.tile_pool(name="ftp", bufs=3) as ftp,
            tc.tile_pool(name="fc2p", bufs=3) as fc2p,
            tc.tile_pool(name="outp", bufs=5) as outp,
        ):
            # ---- persistent activations/weights -----------------------------
            x_t = persist.tile([P, NT, _D], F32)
            xh_t = persist.tile([P, NT, _D], BF16)
            pos_t = persist.tile([P, NT, _D], BF16)
            x1_t = persist.tile([P, NT, _D], F32)
            h_t = persist.tile([P, NT, _D], BF16)
            hT_t = persist.tile([P, KD, _N], BF16)
            Gb = persist.tile([P, KD, _D], BF16)
            sTb = persist.tile([P, KD, _D], BF16)
            swb = [persist.tile([P, _D], BF16, name=f"sw{j}")
                   for j in range(NPAIR)]
            Tb = persist.tile([P, KD, _D], BF16)
            wk_t = persist.tile([P, KD, _D], BF16)
            wqp_t = persist.tile([P, NPAIR, KD, P], BF16)
            m_t = persist.tile([P, _H, KD, _D], BF16)
            cg_t = persist.tile([P, KD, _D], BF16)
            wff1_t = persist.tile([P, KD, _DE], F8)
            wff2_t = persist.tile([P, KE, _D], F8)
            g1T_t = persist.tile([P, KD, _N], F8)
            pb_t = [persist.tile([P, NH], F32, name=f"pb{s}")
                    for s in range(2)]
            sqv_t = [persist.tile([1, NH], BF16, name=f"sqv{s}")
                     for s in range(2)]
            mv0 = persist.tile([P, NT, 2], F32)
            rs0 = persist.tile([P, NT], F32)
            nm0 = persist.tile([P, NT], F32)
            nm1 = persist.tile([P, NT], F32)
            mv1 = persist.tile([P, NT, 2], F32)
            rs1 = persist.tile([P, NT], F32)

            # ---- input DMAs, in consumption order (xh/pos first) ------------
            for t in range(NT):
                nc.sync.dma_start(xh_t[:, t, :], xhr[:, t, :])
            for t in range(NT):
                nc.sync.dma_start(pos_t[:, t, :], posr[:, t, :])
            nc.sync.dma_start(wk_t[:], wk[:, :, :])
            nc.sync.dma_start(m_t[:], m_all[:, :, :, :])
            nc.sync.dma_start(wqp_t[:], wqp[:, :, :, :])
            for t in range(NT):
                nc.sync.dma_start(x_t[:, t, :], xr[:, t, :])
            nc.sync.dma_start(cg_t[:], cg[:, :, :])
            nc.sync.dma_start(wff1_t[:], wff1[:, :, :])
            nc.sync.dma_start(wff2_t[:], wff2[:, :, :])
            if g0b is not None:
                g0_t = persist.tile([P, _D], BF16)
                nc.sync.dma_start(g0_t[:], g0b[:, :])
            if bkr is not None:
                bkr_t = persist.tile([1, _D], BF16)
                nc.sync.dma_start(bkr_t[:], bkr[:, :])
            if bqp is not None:
                bqp_t = persist.tile([P, NPAIR], BF16)
                nc.sync.dma_start(bqp_t[:], bqp[:, :])
            if bmb is not None:
                bm_t = persist.tile([P, _D], F32)
                nc.sync.dma_start(bm_t[:], bmb[:, :])
            if bff1c is not None:
                bff1_t = persist.tile([P, KE], F32)
                nc.sync.dma_start(bff1_t[:], bff1c[:, :])
            if g2c is not None:
                g2_t = persist.tile([P, KE], F32)
                nc.sync.dma_start(g2_t[:], g2c[:, :])
            if b2c is not None:
                b2_t = persist.tile([P, KE], F32)
                nc.sync.dma_start(b2_t[:], b2c[:, :])
            if bf2b is not None:
                bf2_t = persist.tile([P, _D], F32)
                nc.sync.dma_start(bf2_t[:], bf2b[:, :])

            # ---- constants --------------------------------------------------
            warm_t = persist.tile([P, 512], BF16)
            nc.vector.memset(warm_t, 0.001)
            ident_f = persist.tile([P, P], F32)
            make_identity(nc, ident_f)
            ident_b = persist.tile([P, P], BF16)
            nc.vector.tensor_copy(ident_b[:], ident_f[:])
            ones_b = persist.tile([P, 1], BF16)
            nc.vector.memset(ones_b, 1.0)
            ones_8 = persist.tile([P, 1], F8)
            nc.vector.memset(ones_8, 1.0)
            ones1_b = persist.tile([1, P], BF16)
            nc.vector.memset(ones1_b, 1.0)
            eps_t = persist.tile([P, 1], F32)
            nc.vector.memset(eps_t, _EPS)
            eps64_t = persist.tile([P, 1], F32)
            nc.vector.memset(eps64_t, _EPS / 64.0)
            eps65536_t = persist.tile([P, 1], F32)
            nc.vector.memset(eps65536_t, _EPS * 65536.0)

            def filler(n_mm, name):
                """Low-priority PE chains the scheduler drops into idle slots
                to keep the PE activity monitor (clock) up."""
                ps = pmm.tile([P, 512], F32, tag="mm", name=name)
                for w in range(n_mm):
                    mm(ps[:], warm_t[:, :128], warm_t[:],
                       start=(w == 0), stop=(w == n_mm - 1))
                nc.scalar.copy(warm_t[:, 0:1], ps[:, 0:1])

            filler(12, "warm0")

            # ---- LN0 stats (per chunk as x lands), batched rstd -------------
            for t in range(NT):
                st = lnp.tile([P, 6], F32, tag="st")
                nc.vector.bn_stats(st[:], xh_t[:, t, :])
                nc.vector.bn_aggr(mv0[:, t, :], st[:])
            nc.scalar.activation(rs0[:], mv0[:, :, 1], Act.Sqrt,
                                 bias=eps_t[:])
            nc.vector.reciprocal(rs0[:], rs0[:])
            nc.vector.scalar_tensor_tensor(nm0[:], mv0[:, :, 0], -1.0,
                                           rs0[:], op0=Alu.mult,
                                           op1=Alu.mult)

            # ---- LN0 apply + pos + swish -> h; hT; G accumulation -----------
            gps = [pmm.tile([P, 512], F32, tag="acc", bufs=4, name=f"gacc{c}")
                   for c in range(KD)]
            for t in range(NT):
                tmp = lnp.tile([P, _D], BF16, tag="tmp")
                if g0b is None:
                    # z1 = x*rstd + pos; -mu*rstd folded into the silu bias
                    nc.vector.scalar_tensor_tensor(
                        tmp[:], xh_t[:, t, :], rs0[:, t:t + 1], pos_t[:, t, :],
                        op0=Alu.mult, op1=Alu.add)
                else:
                    nc.vector.tensor_scalar(
                        tmp[:], xh_t[:, t, :], mv0[:, t, 0:1],
                        rs0[:, t:t + 1], op0=Alu.subtract, op1=Alu.mult)
                    nc.vector.tensor_mul(tmp[:], tmp[:], g0_t[:])
                    nc.vector.tensor_add(tmp[:], tmp[:], pos_t[:, t, :])
                if g0b is None:
                    nc.scalar.activation(h_t[:, t, :], tmp[:], Act.Silu,
                                         bias=nm0[:, t:t + 1])
                else:
                    nc.scalar.activation(h_t[:, t, :], tmp[:], Act.Silu)
                pt = pmm.tile([P, KD * P], BF16, tag="mm", name="ptT")
                for c in range(KD):
                    nc.tensor.transpose(
                        pt[:, c * P:(c + 1) * P],
                        h_t[:, t, c * P:(c + 1) * P], ident_b[:])
                nc.vector.tensor_copy(
                    hT_t[:, :, t * P:(t + 1) * P],
                    pt[:].rearrange("p (c n) -> p c n", n=P))
                for c in range(KD):
                    mm(gps[c][:], h_t[:, t, c * P:(c + 1) * P],
                       h_t[:, t, :], start=(t == 0), stop=(t == NT - 1))
            # r = 1^T h (only needed for the bk rank-1 term)
            if bkr is not None:
                rps = pmm.tile([1, 512], F32, tag="mm", name="rrow")
                for t in range(NT):
                    mm(rps[:], ones_b[:], h_t[:, t, :],
                       start=(t == 0), stop=(t == NT - 1))
                r_sb = persist.tile([1, _D], BF16)
                nc.scalar.copy(r_sb[:], rps[:])
            for c in range(KD):
                nc.scalar.copy(Gb[:, c, :], gps[c][:])

            # ---- sT = G @ Wk (+ r^T bk) -------------------------------------
            for c in range(KD):
                ps = pmm.tile([P, 512], F32, tag="mm", name="psT")
                last = KD - 1 if bkr is None else KD
                for k in range(KD):
                    mm(ps[:], Gb[:, k, c * P:(c + 1) * P], wk_t[:, k, :],
                       start=(k == 0), stop=(k == last))
                if bkr is not None:
                    mm(ps[:], r_sb[:, c * P:(c + 1) * P], bkr_t[:],
                       start=False, stop=True)
                nc.scalar.copy(sTb[:, c, :], ps[:])

            # ---- sW_h = s_h @ M_h, pairs stacked ----------------------------
            for h_idx in range(_H):
                pw = pmm.tile([P, 512], F32, tag="mm", name="pw")
                for k in range(KD):
                    mm(pw[:64, :], sTb[:, k, h_idx * 64:(h_idx + 1) * 64],
                       m_t[:, h_idx, k, :],
                       start=(k == 0), stop=(k == KD - 1))
                lo = 64 * (h_idx % 2)
                nc.scalar.copy(swb[h_idx // 2][lo:lo + 64, :], pw[:64, :])

            # ---- T = sum_h Wq_h sW_h ----------------------------------------
            for c in range(KD):
                ps = pmm.tile([P, 512], F32, tag="mm", name="pT")
                for pr in range(NPAIR):
                    mm(ps[:], wqp_t[:, pr, c, :], swb[pr][:],
                       start=(pr == 0), stop=(pr == NPAIR - 1))
                nc.scalar.copy(Tb[:, c, :], ps[:])
            if bqp is not None:
                vps = pmm.tile([1, 512], F32, tag="mm", name="vrow")
                for pr in range(NPAIR):
                    mm(vps[:], bqp_t[:, pr:pr + 1], swb[pr][:],
                       start=(pr == 0), stop=(pr == NPAIR - 1))
                v_sb = persist.tile([1, _D], BF16)
                nc.scalar.copy(v_sb[:], vps[:])

            filler(8, "warmA")

            # ---- merge: x1 = x + h @ T (+ 1 (x) v) (+ b_merge) --------------
            for t in range(NT):
                pm = pmm.tile([P, 512], F32, tag="mm", name="pmg")
                last = KD - 1 if bqp is None else KD
                for c in range(KD):
                    mm(pm[:], hT_t[:, c, t * P:(t + 1) * P], Tb[:, c, :],
                       start=(c == 0), stop=(c == last))
                if bqp is not None:
                    mm(pm[:], ones1_b[:], v_sb[:], start=False, stop=True)
                x1c = x1_t[:, t, :]
                nc.vector.tensor_add(x1c, pm[:], x_t[:, t, :])
                if bmb is not None:
                    nc.vector.tensor_add(x1c, x1c, bm_t[:])

            # ---- LN1 stats + batched rstd (x8 fp8 scale folded) -------------
            for t in range(NT):
                st = lnp.tile([P, 6], F32, tag="st")
                nc.vector.bn_stats(st[:], x1_t[:, t, :])
                nc.vector.bn_aggr(mv1[:, t, :], st[:])
            # rs1 = 8 / sqrt(var + eps) = 1 / sqrt((var + eps)/64)
            nc.scalar.activation(rs1[:], mv1[:, :, 1], Act.Sqrt,
                                 bias=eps64_t[:], scale=1.0 / 64.0)
            nc.vector.reciprocal(rs1[:], rs1[:])
            nc.vector.scalar_tensor_tensor(nm1[:], mv1[:, :, 0], -1.0,
                                           rs1[:], op0=Alu.mult,
                                           op1=Alu.mult)

            # ---- g1 (fp8, x8) + transpose -> g1T; z-chain per half ----------
            def g1_apply(t):
                g1c = lnp.tile([P, _D], BF16, tag="g1c", name="g1c")
                nc.vector.tensor_scalar(
                    g1c[:], x1_t[:, t, :], rs1[:, t:t + 1], nm1[:, t:t + 1],
                    op0=Alu.mult, op1=Alu.add,
                )
                pt = pmm.tile([P, KD * P], BF16, tag="mm", name="ptG")
                for c in range(KD):
                    nc.tensor.transpose(
                        pt[:, c * P:(c + 1) * P],
                        g1c[:, c * P:(c + 1) * P], ident_b[:])
                nc.scalar.copy(
                    g1T_t[:, :, t * P:(t + 1) * P],
                    pt[:].rearrange("p (c n) -> p c n", n=P))

            # LN2 rstd via Gram trick: ssq_n = g1_n^T C g1_n — needs only this
            # half's g1T, so rstd is ready before the half's swishes start
            def zchain(s):
                psq_s = pmm.tile([1, NH], F32, tag="mm", name=f"psq{s}")
                for c in range(KD):
                    zps = pmm.tile([P, NH], F32, tag="mm", name="zps")
                    for k in range(KD):
                        mm(zps[:], cg_t[:, k, c * P:(c + 1) * P],
                           g1T_t[:, k, s * NH:(s + 1) * NH],
                           start=(k == 0), stop=(k == KD - 1))
                    prod = sqp.tile([P, NH], BF16, tag="sq", name="prod")
                    nc.vector.tensor_tensor(
                        prod[:], zps[:],
                        g1T_t[:, c, s * NH:(s + 1) * NH], op=Alu.mult)
                    mm(psq_s[:], ones_b[:], prod[:],
                       start=(c == 0), stop=(c == KD - 1))
                return psq_s

            for t in range(4):
                g1_apply(t)
            psq = [zchain(0), None]
            for t in range(4, NT):
                g1_apply(t)

            filler(8, "warmB")

            # ---- ff1 -> fc2 (straight from PSUM) -> swish -> ff2 -> y -------
            for s in range(2):
                if s == 1:
                    psq[1] = zchain(1)
                # finalize this half's rstd just-in-time (keeps the scalar
                # queue free of cross-half stat ops ahead of the swishes)
                # sqv2 = 256*sqrt(ssq_raw/(64*DE) + eps); pb = rstd2/256
                nc.scalar.activation(sqv_t[s][:, :], psq[s][:], Act.Sqrt,
                                     bias=eps65536_t[:1, :],
                                     scale=65536.0 / (64.0 * _DE))
                pbb = pmm.tile([P, NH], F32, tag="mm", name=f"pbb{s}")
                mm(pbb[:], ones1_b[:], sqv_t[s][:, :], start=True, stop=True)
                nc.vector.reciprocal(pb_t[s][:], pbb[:])
                pos_ = [pmm.tile([P, 512], F32, tag="acc", bufs=4,
                                 name=f"po{s}_{tt}") for tt in range(4)]
                for o in range(KE):
                    pf = pmm.tile([P, NH], F32, tag="mm", name="pf")
                    for c in range(2):
                        mm(pf[:],
                           wff1_t[:, 2 * c:2 * c + 2, o * P:(o + 1) * P],
                           g1T_t[:, 2 * c:2 * c + 2, s * NH:(s + 1) * NH],
                           start=(c == 0), stop=(c == 1), perf_mode=DR)
                    if o % 2 == 0:
                        fc2 = fc2p.tile([P, 2, NH], BF16, tag="fc2")
                        fT2 = ftp.tile([P, 2, NH], F8, tag="ft", name="fT2")
                    # fc2 = (256*fc) * (rstd2/256) = fc * rstd2
                    nc.vector.tensor_tensor(fc2[:, o % 2, :], pf[:],
                                            pb_t[s][:], op=Alu.mult)
                    if g2c is not None or b2c is not None:
                        # per-o swish so the per-DE-block gain/bias applies
                        akw = {}
                        if g2c is not None:
                            akw["scale"] = g2_t[:, o:o + 1]
                        if b2c is not None:
                            akw["bias"] = b2_t[:, o:o + 1]
                        nc.scalar.activation(fT2[:, o % 2, :],
                                             fc2[:, o % 2, :],
                                             Act.Silu, **akw)
                    elif o % 2 == 1:
                        nc.scalar.activation(
                            fT2[:].rearrange("p a n -> p (a n)"),
                            fc2[:].rearrange("p a n -> p (a n)"),
                            Act.Silu)
                    if o % 2 == 1:
                        oc = o // 2
                        for tt in range(4):
                            mm(pos_[tt][:],
                               fT2[:, :, tt * P:(tt + 1) * P],
                               wff2_t[:, 2 * oc:2 * oc + 2, :],
                               start=(oc == 0), stop=(oc == KE // 2 - 1),
                               perf_mode=DR)
                # y = x1 + psum/32 (+ b_ff2)
                for tt in range(4):
                    t = s * 4 + tt
                    oc_ = outp.tile([P, _D], F32, tag="oc")
                    nc.vector.scalar_tensor_tensor(
                        oc_[:], pos_[tt][:], 1.0 / 32.0, x1_t[:, t, :],
                        op0=Alu.mult, op1=Alu.add)
                    if bf2b is not None:
                        nc.vector.tensor_add(oc_[:], oc_[:], bf2_t[:])
                    nc.sync.dma_start(yr[:, t, :], oc_[:])

    _split_excess_waits(nc)
    return nc, needed


def _host_fold(inputs):
    """Precompute weight layouts/folds. Returns (arrays, flags)."""
    import ml_dtypes
    f32 = np.float32
    bf16 = ml_dtypes.bfloat16
    f8 = ml_dtypes.float8_e4m3

    W_qkv = np.asarray(inputs["W_qkv"], f32)
    b_qkv = np.asarray(inputs["b_qkv"], f32)
    W_merge = np.asarray(inputs["W_merge"], f32)
    alpha = float(np.asarray(inputs["scale"])) ** -0.5

    P = _P

    def col128(w):  # (D, F) -> (128, D//128, F), d = c*128 + p
        d, f = w.shape
        return np.ascontiguousarray(w.reshape(d // P, P, f).transpose(1, 0, 2))

    def colvec(v):  # (F,) -> (128, F//128), f = o*128 + p
        return np.ascontiguousarray(v.reshape(-1, P).T)

    def bcast(v):  # (D,) -> (128, D)
        return np.ascontiguousarray(np.broadcast_to(v, (P, v.shape[0])))

    Wq = np.ascontiguousarray(W_qkv[:, :_D]) * f32(alpha)
    Wk = np.ascontiguousarray(W_qkv[:, _D:2 * _D])
    bq = b_qkv[:_D] * f32(alpha)
    bk = b_qkv[_D:2 * _D]
    bv = b_qkv[2 * _D:]
    if not np.all(bv == 0.0):
        raise NotImplementedError("nonzero v-bias not supported by the fold")
    Wv = W_qkv[:, 2 * _D:].reshape(_D, _H, _D)

    M = np.empty((P, _H, _KD, _D), bf16)
    Wm64 = W_merge.astype(np.float64).reshape(_H, _D, _D)
    for h in range(_H):
        mh = (Wv[:, h, :].astype(np.float64) @ Wm64[h]).astype(f32)
        M[:, h] = col128(mh).astype(bf16)

    # Wq pairs, transposed: (p, pr, c, i) = Wq[c*128+i, pr*128+p]
    wqp = np.ascontiguousarray(
        Wq.reshape(_KD, P, _H // 2, P).transpose(3, 2, 0, 1)).astype(bf16)

    ln0_g = np.asarray(inputs["ln0_g"], f32)
    ln1_g = np.asarray(inputs["ln1_g"], np.float64)
    ln1_b = np.asarray(inputs["ln1_b"], np.float64)
    W_ff1 = np.asarray(inputs["W_ff1"], np.float64)
    w1 = ln1_g[:, None] * W_ff1
    b1 = np.asarray(inputs["b_ff1"], np.float64) + ln1_b @ W_ff1
    # Center so the ff1 matmul emits LN2-pre-centered activations
    w1c = (w1 - w1.mean(axis=1, keepdims=True)).astype(f32)
    b1c = (b1 - b1.mean()).astype(f32)
    if not np.all(b1c == 0.0):
        raise NotImplementedError(
            "nonzero centered ff1 bias breaks the LN2 Gram-trick sumsq")

    b_merge = np.asarray(inputs["b_merge"], f32)
    b_ff2 = np.asarray(inputs["b_ff2"], f32)
    ln2_g = np.asarray(inputs["ln2_g"], f32)
    ln2_b = np.asarray(inputs["ln2_b"], f32)

    pos2 = (np.asarray(inputs["pos_enc"], f32).reshape(_N, _D)
            + np.asarray(inputs["ln0_b"], f32))

    flags = {
        "g0": bool(np.all(ln0_g == 1.0)),
        "bq": bool(np.all(bq == 0.0)),
        "bk": bool(np.all(bk == 0.0)),
        "bm": bool(np.all(b_merge == 0.0)),
        "bff1": bool(np.all(b1c == 0.0)),
        "g2": bool(np.all(ln2_g == 1.0)),
        "b2": bool(np.all(ln2_b == 0.0)),
        "bf2": bool(np.all(b_ff2 == 0.0)),
    }

    arrays = {
        "pos2": np.ascontiguousarray(pos2).astype(bf16),
        "cg": col128((w1c.astype(np.float64) @ w1c.astype(np.float64).T
                      ).astype(f32)).astype(bf16),
        "g0b": bcast(ln0_g).astype(bf16),
        "wk": col128(Wk).astype(bf16),
        "wqp": wqp,
        "m_all": M,
        "bkr": np.ascontiguousarray(bk[None, :]).astype(bf16),
        "bqp": np.ascontiguousarray(bq.reshape(_H // 2, P).T).astype(bf16),
        "bmb": bcast(b_merge),
        "wff1": col128(w1c * 32.0).astype(f8),
        "bff1c": colvec(b1c),
        "g2c": colvec(ln2_g),
        "b2c": colvec(ln2_b),
        "wff2": col128(np.asarray(inputs["W_ff2"], f32) * 32.0).astype(f8),
        "bf2b": bcast(b_ff2),
    }
    return arrays, flags


_PROGRAM_CACHE = {}


def _get_program(flags):
    key = tuple(sorted(flags.items()))
    if key not in _PROGRAM_CACHE:
        _PROGRAM_CACHE[key] = _build_program(flags)
    return _PROGRAM_CACHE[key]


def _in_maps(inputs, arrays, needed):
    import ml_dtypes as _ml
    x = np.asarray(inputs["x"], np.float32)
    shared = {k: arrays[k] for k in needed if k not in ("xb", "xbh")}
    in_maps = []
    for core in range(_NCORES):
        m = dict(shared)
        xc = np.ascontiguousarray(x[core])
        m["xb"] = xc
        if "xbh" in needed:
            m["xbh"] = xc.astype(_ml.bfloat16)
        in_maps.append(m)
    return in_maps


def kernel(**inputs):
    from concourse.bass_utils import run_bass_kernel_spmd

    arrays, flags = _host_fold(inputs)
    nc, needed = _get_program(flags)
    in_maps = _in_maps(inputs, arrays, needed)
    res = run_bass_kernel_spmd(nc, in_maps, core_ids=list(range(_NCORES)))
    out = np.stack([r["y"] for r in res.results], axis=0)
    return out.astype(np.float32)
